# revision 1
# baseline (speedup 1.0000x reference)
"""MDTA (Restormer transposed attention) Trainium2 Bass kernel.

Strategy: data-parallel over batch (8 cores x 1 image each). Per core:
  1. qkv 1x1 conv as bf16 GEMM (PE) from an SBUF-resident bf16 copy of x,
     channel-chunked (5 M-chunks of <=128), K=64 tails row-packed in pairs.
  2. depthwise 3x3 conv as 9 diagonal-lhsT matmuls per chunk-tile accumulated
     in PSUM (PE), with AP-clipped image-edge taps + strided DVE fixes for
     row-wrap contamination of the dx=+-1 taps.
  3. q,k: L2 norms (ACT Square+accum), scale q_hat=temp*q/||q||, k_hat=k/||k||
     in channel-part layout, PE-transpose 128x128 blocks to spatial-part
     layout, attention q_hat^T k_hat accumulated over 128 spatial blocks.
  4. block-diag mask + softmax -> A; fold output projection: M^T = A^T w_out^T
     (small PE matmuls); y = M^T.T @ v_dw as a single GEMM over v.
"""
import sys
sys.path.insert(0, "/opt/trn_rl_repo")
from contextlib import ExitStack

import numpy as np
import ml_dtypes

import concourse.bass as bass
import concourse.mybir as mybir
import concourse.tile as tile
from concourse import bacc
from concourse.bass_utils import run_bass_kernel_spmd

F32 = mybir.dt.float32
F32R = mybir.dt.float32r
BF16 = mybir.dt.bfloat16
AF = mybir.ActivationFunctionType
ALU = mybir.AluOpType
AX = mybir.AxisListType

C = 192
NHEADS = 8
HDIM = 24
H = W = 128
N = H * W            # 16384 spatial positions
NT = 512             # free-dim tile (4 image rows)
NTILES = N // NT     # 32
EPS = 1e-12
# output-channel chunks of the 576-row qkv: (start, width, role)
# roles: q = 0..191, k = 192..383, v = 384..575
CHUNKS = [(0, 128), (128, 128), (256, 128), (384, 128), (512, 64)]
TAPS = [(dy, dx) for dy in (-1, 0, 1) for dx in (-1, 0, 1)]
TAP_ORDER = [4, 0, 1, 2, 3, 5, 6, 7, 8]  # center tap first (start=True, full width)


def build_nc(reps=1, abl=()):  # noqa: C901
    nc = bacc.Bacc("TRN2", target_bir_lowering=False, debug=False)
    x_d = (nc.dram_tensor("x_scratch", [C, N], F32) if "dummyx" in abl
           else nc.dram_tensor("x", [C, N], F32, kind="ExternalInput"))
    wq_d = nc.dram_tensor("w_qkvT", [C, 576], BF16, kind="ExternalInput")
    wdd_d = nc.dram_tensor("w_dwd", [5, 128, 9 * 128], BF16, kind="ExternalInput")
    wdn_d = nc.dram_tensor("w_dwn", [5, 128, 9], F32, kind="ExternalInput")  # negated taps
    wdp_d = nc.dram_tensor("w_dwp", [5, 128, 9], F32, kind="ExternalInput")  # taps
    wo_d = nc.dram_tensor("w_outT", [C, C], F32, kind="ExternalInput")
    tmp_d = nc.dram_tensor("temp", [C, 1], F32, kind="ExternalInput")
    mask_d = nc.dram_tensor("mask", [C, C], F32, kind="ExternalInput")
    eye_d = nc.dram_tensor("eye", [128, 128], BF16, kind="ExternalInput")
    if "dummyy" in abl:
        y_d = nc.dram_tensor("y_scratch", [C, N], F32)
        yprobe_d = nc.dram_tensor("y", [128, 4], F32, kind="ExternalOutput")
    else:
        y_d = nc.dram_tensor("y", [C, N], F32, kind="ExternalOutput")
        yprobe_d = None

    with tile.TileContext(nc) as tc, ExitStack() as ctx:
        wp = ctx.enter_context(tc.tile_pool(name="w", bufs=1))
        prep = ctx.enter_context(tc.tile_pool(name="pre", bufs=1))
        dwdp = ctx.enter_context(tc.tile_pool(name="dwd", bufs=1))
        sp = ctx.enter_context(tc.tile_pool(name="small", bufs=1))
        yp = ctx.enter_context(tc.tile_pool(name="y", bufs=2))
        gps = ctx.enter_context(tc.tile_pool(name="gps", bufs=2, space="PSUM"))
        dps = ctx.enter_context(tc.tile_pool(name="dps", bufs=2, space="PSUM"))

        # ---- persistent weights ----
        wq0 = wp.tile([128, 576], BF16, tag="wq0")
        wq1 = wp.tile([128, 576], BF16, tag="wq1")  # ch 128..191 duplicated
        nc.sync.dma_start(wq0[:], wq_d[0:128, :])
        nc.sync.dma_start(wq1[0:64, :], wq_d[128:192, :])
        nc.sync.dma_start(wq1[64:128, :], wq_d[128:192, :])
        wo0 = wp.tile([128, C], F32, tag="wo0")
        wo1 = wp.tile([64, C], F32, tag="wo1")
        nc.sync.dma_start(wo0[:], wo_d[0:128, :])
        nc.sync.dma_start(wo1[:], wo_d[128:192, :])
        eye_s = wp.tile([128, 128], BF16, tag="eye")
        nc.sync.dma_start(eye_s[:], eye_d[:])
        tmp0 = wp.tile([128, 1], F32, tag="tmp0")
        tmp1 = wp.tile([64, 1], F32, tag="tmp1")
        nc.sync.dma_start(tmp0[:], tmp_d[0:128, :])
        nc.sync.dma_start(tmp1[:], tmp_d[128:192, :])
        mask0 = wp.tile([128, C], F32, tag="mask0")
        mask1 = wp.tile([64, C], F32, tag="mask1")
        nc.sync.dma_start(mask0[:], mask_d[0:128, :])
        nc.sync.dma_start(mask1[:], mask_d[128:192, :])
        # M^T (built later, used in phase 2)
        mt0 = wp.tile([128, C], BF16, tag="mt0")
        mt1 = wp.tile([64, C], BF16, tag="mt1")

        # resident bf16 copy of x (loaded once; GEMM reads SBUF, PE never
        # stalls on HBM)
        xres0 = wp.tile([128, N], BF16, tag="xres0")
        xres1 = wp.tile([128, N], BF16, tag="xres1")  # ch 128..191 duplicated on parts 64..127
        with tc.tile_pool(name="xload", bufs=3) as xp:
            for t in range(NTILES):
                cols = slice(t * NT, (t + 1) * NT)
                xt0 = xp.tile([128, NT], F32, tag="x0")
                xt1 = xp.tile([64, NT], F32, tag="x1")
                nc.sync.dma_start(xt0[:], x_d[0:128, cols])
                nc.sync.dma_start(xt1[:], x_d[128:192, cols])
                nc.vector.tensor_copy(xres0[:, cols], xt0[:])
                nc.vector.tensor_copy(xres1[0:64, cols], xt1[:])
                nc.vector.tensor_copy(xres1[64:128, cols], xt1[:])

        def gemm_chunk(ci, dst, dst_row0=0):
            """qkv GEMM for chunk ci into dst[dst_row0:dst_row0+mw, :] (bf16)."""
            mc0, mw = CHUNKS[ci]
            for tp in range(NTILES // 2):
                pgs = []
                for half in (0, 1):
                    t = 2 * tp + half
                    cols = slice(t * NT, (t + 1) * NT)
                    pg = gps.tile([mw, NT], F32, tag="g")
                    pgs.append((t, cols, pg))
                    nc.tensor.matmul(pg[:], wq0[:, mc0:mc0 + mw],
                                     xres0[:, cols], start=True, stop=False)
                # the two K=64 tails run concurrently on disjoint row strips
                for half in (0, 1):
                    t, cols, pg = pgs[half]
                    p0 = 64 * half
                    nc.tensor.matmul(pg[:],
                                     wq1[p0:p0 + 64, mc0:mc0 + mw],
                                     xres1[p0:p0 + 64, cols],
                                     start=False, stop=True,
                                     tile_position=(p0, 0))
                for t, cols, pg in pgs:
                    nc.vector.tensor_copy(
                        dst[dst_row0:dst_row0 + mw, cols], pg[:])

        def load_dwd(ci):
            mw = CHUNKS[ci][1]
            dwd = dwdp.tile([128, 9 * 128], BF16, tag="dwd")
            nc.sync.dma_start(dwd[:], wdd_d[ci])
            dwn = dwdp.tile([128, 9], F32, tag="dwn")
            nc.sync.dma_start(dwn[:], wdn_d[ci])
            dwpos = dwdp.tile([128, 9], F32, tag="dwpos")
            nc.sync.dma_start(dwpos[:], wdp_d[ci])
            return dwd, dwn, dwpos

        def dwconv_chunk(ci, pre, dwd, dst, dst_row0=0, engine="pe", dwp_=None):
            """Depthwise 3x3 of pre (mw,N) bf16 -> dst[dst_row0:...] bf16."""
            mc0, mw = CHUNKS[ci]
            for t in range(NTILES):
                base = t * NT
                if engine == "pe":
                    pd = dps.tile([mw, NT], F32, tag="d")
                    first = True
                    for ti in (TAP_ORDER[:1] if "tap1" in abl else TAP_ORDER):
                        dy, dx = TAPS[ti]
                        sh = dy * W + dx
                        lo = max(0, -(base + sh))
                        hi = min(NT, N - base - sh)
                        nc.tensor.matmul(
                            pd[:, lo:hi],
                            dwd[0:mw, ti * 128:ti * 128 + mw],
                            pre[0:mw, base + sh + lo:base + sh + hi],
                            start=first, stop=(first if "tap1" in abl else (ti == TAP_ORDER[-1])))
                        first = False
                    nc.scalar.copy(
                        dst[dst_row0:dst_row0 + mw, base:base + NT], pd[:])
                else:
                    # DVE: 9 scalar_tensor_tensor FMAs straight into dst (bf16)
                    dcols = slice(dst_row0, dst_row0 + mw)
                    out = dst[dcols, base:base + NT]
                    first = True
                    for ti in (TAP_ORDER[:1] if "tap1" in abl else TAP_ORDER):
                        dy, dx = TAPS[ti]
                        sh = dy * W + dx
                        lo = max(0, -(base + sh))
                        hi = min(NT, N - base - sh)
                        src = pre[0:mw, base + sh + lo:base + sh + hi]
                        wcol = dwp_[0:mw, ti:ti + 1]
                        if first:
                            nc.vector.tensor_scalar_mul(out, src, wcol)
                            first = False
                        else:
                            nc.vector.scalar_tensor_tensor(
                                out=dst[dcols, base + lo:base + hi], in0=src,
                                scalar=wcol,
                                in1=dst[dcols, base + lo:base + hi],
                                op0=ALU.mult, op1=ALU.add)

        def edge_fixes(ci, pre, dwn, dst, dst_row0=0):
            """Subtract row-wrap contamination of dx=+-1 taps (strided STT)."""
            mw = CHUNKS[ci][1]
            for dy in (-1, 0, 1):
                ti_l = (dy + 1) * 3 + 0   # (dy, dx=-1)
                y0, y1 = max(0, 1 - dy), min(127, 128 - dy)
                out_ap = dst[dst_row0:dst_row0 + mw, y0 * W:y1 * W + 1:W]
                src_ap = pre[0:mw, (y0 + dy) * W - 1:(y1 + dy) * W:W]
                nc.vector.scalar_tensor_tensor(
                    out=out_ap, in0=src_ap, scalar=dwn[0:mw, ti_l:ti_l + 1],
                    in1=out_ap, op0=ALU.mult, op1=ALU.add)
                ti_r = (dy + 1) * 3 + 2   # (dy, dx=+1)
                y0, y1 = max(0, -1 - dy), min(127, 126 - dy)
                out_ap = dst[dst_row0:dst_row0 + mw,
                             y0 * W + W - 1:y1 * W + W:W]
                src_ap = pre[0:mw, (y0 + dy + 1) * W:(y1 + dy + 1) * W + 1:W]
                nc.vector.scalar_tensor_tensor(
                    out=out_ap, in0=src_ap, scalar=dwn[0:mw, ti_r:ti_r + 1],
                    in1=out_ap, op0=ALU.mult, op1=ALU.add)

        # ================= phase 1: q,k =================
        for _rep in range(reps):
         with tc.tile_pool(name="kT", bufs=1) as kTp, \
             tc.tile_pool(name="dwout", bufs=1) as dwp, \
             tc.tile_pool(name="junk", bufs=1) as jp, \
             tc.tile_pool(name="qt", bufs=3) as qtp, \
             tc.tile_pool(name="asb", bufs=1) as ap_, \
             tc.tile_pool(name="tps", bufs=2, space="PSUM") as tps, \
             tc.tile_pool(name="aps", bufs=2, space="PSUM") as aps:

            kT = kTp.tile([128, 128 * C], BF16, tag="kT")  # block b at cols b*192
            junk = jp.tile([128, 2048], BF16, tag="junk")
            stat = sp.tile([128, 8], F32, tag="stat")
            apq0 = aps.tile([128, C], F32, tag="attn")
            apq1 = aps.tile([64, C], F32, tag="attn")

            def norms_scale(ci, dw, is_q_lo, is_k_hi, q_off=None):
                """Compute 1/max(||row||,eps) (*temp for q rows), scale dw."""
                mw = CHUNKS[ci][1]
                for s4 in range(8):
                    nc.scalar.activation(
                        junk[0:mw, :], dw[0:mw, s4 * 2048:(s4 + 1) * 2048],
                        AF.Square, accum_out=stat[0:mw, s4:s4 + 1])
                n2 = sp.tile([128, 1], F32, tag="n2")
                nc.vector.tensor_reduce(n2[0:mw, :], stat[0:mw, 0:8],
                                        axis=AX.X, op=ALU.add)
                nc.scalar.activation(n2[0:mw, :], n2[0:mw, :], AF.Sqrt)
                nc.vector.tensor_scalar_max(n2[0:mw, :], n2[0:mw, :], EPS)
                rs = sp.tile([128, 1], F32, tag="rs")
                nc.vector.reciprocal(rs[0:mw, :], n2[0:mw, :])
                if is_q_lo:  # some leading rows are q channels: multiply temp
                    qw, toff = is_q_lo
                    nc.vector.tensor_tensor(
                        rs[0:qw, :], rs[0:qw, :],
                        (tmp0 if toff < 128 else tmp1)[toff % 128:toff % 128 + qw, :],
                        op=ALU.mult)
                nc.vector.tensor_scalar_mul(dw[0:mw, :], dw[0:mw, :], rs[0:mw, :])

            def transpose_blocks(ci, dw, k_cols=None, q_rows=None):
                """PE-transpose dw blocks; k cols -> kT, q cols -> attn matmuls.

                k_cols: (part_lo, part_hi, kT_off) slice of dw partitions that
                        are k channels; q_rows: (part_lo, part_hi, attn_psum,
                        attn_row0) for q channels.
                """
                mw = CHUNKS[ci][1]
                nblk = 1 if "notr" in abl else 128
                kT3 = kT[:].rearrange("p (blk c) -> p blk c", c=C)
                for b0 in range(0, nblk, 2):
                    npair = min(2, nblk - b0)
                    pt = tps.tile([128, 2 * mw], BF16, tag="t")
                    pt3 = pt[:].rearrange("p (two c) -> p two c", c=mw)
                    for h in range(npair):
                        b = b0 + h
                        nc.tensor.transpose(
                            pt[:, h * mw:(h + 1) * mw],
                            dw[0:mw, b * 128:(b + 1) * 128], eye_s[0:mw, 0:mw])
                    if k_cols is not None:
                        plo, phi, koff = k_cols
                        nc.scalar.copy(
                            kT3[:, b0:b0 + npair, koff:koff + (phi - plo)],
                            pt3[:, 0:npair, plo:phi])
                    if q_rows is not None:
                        plo, phi, apsum, arow0 = q_rows
                        qw = phi - plo
                        qtb = qtp.tile([128, 256], BF16, tag="qt")
                        nc.scalar.copy(qtb[:].rearrange(
                            "p (two c) -> p two c", c=128)[:, 0:npair, 0:qw],
                            pt3[:, 0:npair, plo:phi])
                        for h in range(npair):
                            b = b0 + h
                            nc.tensor.matmul(
                                apsum[arow0:arow0 + qw, :],
                                qtb[:, h * 128:h * 128 + qw],
                                kT[:, b * C:(b + 1) * C],
                                start=(b == 0),
                                stop=(b == (0 if "notr" in abl else 127)),
                                skip_group_check=True)

            # ---- chunk 2: k channels 256..383 (k-local 64..191) ----
            pre = prep.tile([128, N], BF16, tag="pre")
            dwd, dwn, dwpos = load_dwd(2)
            gemm_chunk(2, pre)
            dw = dwp.tile([128, N], BF16, tag="dw")
            dwconv_chunk(2, pre, dwd, dw)
            edge_fixes(2, pre, dwn, dw)
            norms_scale(2, dw, None, None)
            transpose_blocks(2, dw, k_cols=(0, 128, 64))

            # ---- chunk 1: q 128..191 (parts 0..63) + k 192..255 (parts 64..127) ----
            pre = prep.tile([128, N], BF16, tag="pre")
            dwd, dwn, dwpos = load_dwd(1)
            gemm_chunk(1, pre)
            dw = dwp.tile([128, N], BF16, tag="dw")
            dwconv_chunk(1, pre, dwd, dw)
            edge_fixes(1, pre, dwn, dw)
            norms_scale(1, dw, (64, 128), None)
            transpose_blocks(1, dw, k_cols=(64, 128, 0),
                             q_rows=(0, 64, apq1, 0))

            # ---- chunk 0: q channels 0..127 ----
            pre = prep.tile([128, N], BF16, tag="pre")
            dwd, dwn, dwpos = load_dwd(0)
            gemm_chunk(0, pre)
            dw = dwp.tile([128, N], BF16, tag="dw")
            dwconv_chunk(0, pre, dwd, dw)
            edge_fixes(0, pre, dwn, dw)
            norms_scale(0, dw, (128, 0), None)
            transpose_blocks(0, dw, q_rows=(0, 128, apq0, 0))

            # ---- attention: mask + softmax + M^T ----
            def softmax_rows(apsum, msk, mw):
                a = ap_.tile([mw, C], F32, tag=f"a{mw}")
                nc.vector.tensor_tensor(a[:], apsum[:], msk[0:mw, :], op=ALU.add)
                mx = sp.tile([128, 1], F32, tag="mx")
                nc.vector.tensor_reduce(mx[0:mw, :], a[:], axis=AX.X, op=ALU.max)
                nmx = sp.tile([128, 1], F32, tag="nmx")
                nc.vector.tensor_scalar_mul(nmx[0:mw, :], mx[0:mw, :], -1.0)
                nc.scalar.activation(a[:], a[:], AF.Exp, bias=nmx[0:mw, :])
                sm = sp.tile([128, 1], F32, tag="sm")
                nc.vector.tensor_reduce(sm[0:mw, :], a[:], axis=AX.X, op=ALU.add)
                rsm = sp.tile([128, 1], F32, tag="rsm")
                nc.vector.reciprocal(rsm[0:mw, :], sm[0:mw, :])
                nc.vector.tensor_scalar_mul(a[:], a[:], rsm[0:mw, :])
                return a
            a0 = softmax_rows(apq0, mask0, 128)
            a1 = softmax_rows(apq1, mask1, 64)

            # M^T[d,o] = sum_c A[c,d] w_outT[c,o]; K = c (192 -> 2 chunks)
            for dlo, dw_, mt in ((0, 128, mt0), (128, 64, mt1)):
                pm = tps.tile([128, C], F32, tag="t")
                nc.tensor.matmul(pm[0:dw_, :], a0[:, dlo:dlo + dw_],
                                 wo0[:], start=True, stop=False)
                nc.tensor.matmul(pm[0:dw_, :], a1[:, dlo:dlo + dw_],
                                 wo1[:], start=False, stop=True)
                nc.any.tensor_copy(mt[:], pm[0:dw_, :])

         with tc.tile_pool(name="v3", bufs=1) as v3p, \
             tc.tile_pool(name="v4", bufs=1) as v4p:
            v3 = v3p.tile([128, N], BF16, tag="v3")
            v4 = v4p.tile([64, N], BF16, tag="v4")
            for ci, vt in ((3, v3), (4, v4)):
                mw = CHUNKS[ci][1]
                pre = prep.tile([mw, N], BF16, tag="pre")
                dwd, dwn, dwpos = load_dwd(ci)
                gemm_chunk(ci, pre)
                dwconv_chunk(ci, pre, dwd, vt)
                edge_fixes(ci, pre, dwn, vt)

            for t in range(NTILES):
                cols = slice(t * NT, (t + 1) * NT)
                py0 = gps.tile([128, NT], F32, tag="g")
                nc.tensor.matmul(py0[:], mt0[:, 0:128], v3[:, cols],
                                 start=True, stop=False)
                nc.tensor.matmul(py0[:], mt1[:, 0:128], v4[:, cols],
                                 start=False, stop=True)
                y0 = yp.tile([128, NT], F32, tag="y0")
                nc.any.tensor_copy(y0[:], py0[:])
                nc.sync.dma_start(y_d[0:128, cols], y0[:])
                py1 = gps.tile([64, NT], F32, tag="g")
                nc.tensor.matmul(py1[:], mt0[:, 128:192], v3[:, cols],
                                 start=True, stop=False)
                nc.tensor.matmul(py1[:], mt1[:, 128:192], v4[:, cols],
                                 start=False, stop=True)
                y1 = yp.tile([64, NT], F32, tag="y1")
                nc.any.tensor_copy(y1[:], py1[:])
                nc.sync.dma_start(y_d[128:192, cols], y1[:])
            if yprobe_d is not None:
                nc.sync.dma_start(yprobe_d[:], y0[:, 0:4])

    nc.compile()
    return nc


def host_inputs(x, w_qkv, w_dw, w_out, temperature):
    """Host-side prep: per-core input maps."""
    b = x.shape[0]
    w_dw9 = np.asarray(w_dw, np.float32).reshape(576, 9)
    wdd = np.zeros((5, 128, 9 * 128), np.float32)
    wdn = np.zeros((5, 128, 9), np.float32)
    for ci, (s, wid) in enumerate(CHUNKS):
        for t in range(9):
            wdd[ci, :wid, t * 128:t * 128 + wid][np.arange(wid), np.arange(wid)] = \
                w_dw9[s:s + wid, t]
        wdn[ci, :wid, :] = -w_dw9[s:s + wid, :]
    temp_pc = np.repeat(np.asarray(temperature, np.float32).reshape(NHEADS), HDIM
                        ).reshape(C, 1)
    mask = np.full((C, C), -1e9, np.float32)
    for h in range(NHEADS):
        mask[h * HDIM:(h + 1) * HDIM, h * HDIM:(h + 1) * HDIM] = 0.0
    shared = {
        "w_qkvT": np.ascontiguousarray(np.asarray(w_qkv, np.float32).T
                                       ).astype(ml_dtypes.bfloat16),
        "w_dwd": wdd.astype(ml_dtypes.bfloat16),
        "w_dwn": wdn,
        "w_dwp": -wdn,
        "w_outT": np.ascontiguousarray(np.asarray(w_out, np.float32).T),
        "temp": temp_pc,
        "mask": mask,
        "eye": np.eye(128, dtype=ml_dtypes.bfloat16),
    }
    return [dict(shared, x=np.ascontiguousarray(
        np.asarray(x[c], np.float32).reshape(C, N))) for c in range(b)]


_NC_CACHE = {}


def kernel(x, w_qkv, w_dw, w_out, temperature):
    x = np.asarray(x)
    if "nc" not in _NC_CACHE:
        _NC_CACHE["nc"] = build_nc()
    nc = _NC_CACHE["nc"]
    in_maps = host_inputs(x, w_qkv, w_dw, w_out, temperature)
    res = run_bass_kernel_spmd(nc, in_maps, list(range(8)))
    out = np.stack([res.results[c]["y"].reshape(C, H, W) for c in range(8)])
    return out.astype(np.float32)



# revision 25
# speedup vs baseline: 2.4187x; 2.4187x over previous
"""MDTA (Restormer transposed attention) Trainium2 Bass kernel.

Data-parallel over batch (8 cores x 1 image). Per core:
  1. qkv 1x1 conv as bf16 GEMM (PE); q,k chunk outputs stored fp8e4,
     v chunks bf16.
  2. depthwise 3x3 conv: q,k chunks via fp8 DoubleRow diagonal matmuls
     (2 taps per matmul, strided pair APs, 2x PE throughput); interior
     tiles use 5 pair-matmuls, edge tiles 9 singles with AP clipping.
     v chunks via bf16 diagonal matmuls. Row-wrap contamination of
     dx=+-1 taps fixed with strided DVE STT ops.
  3. q,k kept UNNORMALIZED in fp8: PE-transpose (stride-2 fp8 out) to
     spatial-part layout, Gram G = q^T k via fp8 DoubleRow over block
     pairs. Normalization applied to the 192x192 logits instead:
     row scale temp/||q|| (per-partition) and column scale 1/||k||
     (broadcast tile built by a K=1 matmul from PE-transposed norms).
  4. blockdiag mask + softmax -> A; M^T = A^T w_out^T; y = M^T.T @ v.
"""
import sys
sys.path.insert(0, "/opt/trn_rl_repo")
from contextlib import ExitStack

import numpy as np
import ml_dtypes

import concourse.bass as bass
import concourse.mybir as mybir
import concourse.tile as tile
from concourse import bacc
from concourse.bass_utils import run_bass_kernel_spmd

F32 = mybir.dt.float32
BF16 = mybir.dt.bfloat16
F8 = mybir.dt.float8e4
DR = mybir.MatmulPerfMode.DoubleRow
AF = mybir.ActivationFunctionType
ALU = mybir.AluOpType
AX = mybir.AxisListType

C = 192
NHEADS = 8
HDIM = 24
H = W = 128
N = H * W            # 16384 spatial positions
NT = 512             # free-dim tile (4 image rows)
NTILES = N // NT     # 32
P = W + 1            # padded row pitch for q,k chunks (zero col kills wrap)
NP = H * P           # padded row-major size
NT2 = 2 * P          # dw tile: 2 image rows (psum bank fits 258 f32)
EPS = 1e-12
# output-channel chunks of the 576-row qkv: q = 0..191, k = 192..383,
# v = 384..575
CHUNKS = [(0, 128), (128, 128), (256, 128), (384, 128), (512, 64)]
TAPS = [(dy, dx) for dy in (-1, 0, 1) for dx in (-1, 0, 1)]
TAP_ORDER = [4, 0, 1, 2, 3, 5, 6, 7, 8]  # center tap first (start=True)
# fp8 tap pairs: (tapA, tapB, rhs offset, rhs stride); tapB None => zero
PAIRS = [(4, None, 0, 1),          # center first: start=True
         (0, 2, -P - 1, 2),       # (-1,-1) + (-1,+1)
         (3, 5, -1, 2),           # (0,-1)  + (0,+1)
         (6, 8, P - 1, 2),        # (+1,-1) + (+1,+1)
         (1, 7, -P, 2 * P)]       # (-1,0)  + (+1,0)
# single-tap column (x128) in the wdp8 pair layout
STCOL = {}
for _p, (_a, _b, _o, _s) in enumerate(PAIRS):
    STCOL[_a] = 2 * _p
    if _b is not None:
        STCOL[_b] = 2 * _p + 1


def _pair_ap(t, off, stride, width=NT):
    """rhs AP [parts][2, stride][width, 1] at element offset off."""
    a = t[:]
    return bass.AP(a.tensor, a.offset + off,
                   [list(a.ap[0]), [stride, 2], [1, width]])


def _str2_ap(t, off, n):
    """fp8 transpose out AP: [parts][n, step 2] at element offset off."""
    a = t[:]
    return bass.AP(a.tensor, a.offset + off, [list(a.ap[0]), [2, n]])


def _str3_ap(t, npair, m):
    """read-back AP over stride-2 transpose pairs: [parts][npair, 256][m, 2]."""
    a = t[:]
    return bass.AP(a.tensor, a.offset, [list(a.ap[0]), [256, npair], [2, m]])


def build_nc(reps=1, abl=()):  # noqa: C901
    nc = bacc.Bacc("TRN2", target_bir_lowering=False, debug=False)
    x_d = (nc.dram_tensor("x_scratch", [C, N], BF16) if "dummyx" in abl
           else nc.dram_tensor("x", [C, N], BF16, kind="ExternalInput"))
    wq_d = nc.dram_tensor("w_qkvT", [C, 576], BF16, kind="ExternalInput")
    x8_d = (nc.dram_tensor("x8_scratch", [128, N + N // 2], F8) if "dummyx"
            in abl else nc.dram_tensor("x8", [128, N + N // 2], F8,
                                       kind="ExternalInput"))
    wq8_d = nc.dram_tensor("wq8", [2, 128, 2 * 576], F8, kind="ExternalInput")
    wdp8_d = nc.dram_tensor("w_dwp8", [3, 128, 10 * 128], F8, kind="ExternalInput")
    wdd_d = nc.dram_tensor("w_dwd", [2, 128, 9 * 128], BF16, kind="ExternalInput")
    wdb_d = nc.dram_tensor("w_dwb", [128, 6 * 128], BF16, kind="ExternalInput")
    wdn_d = nc.dram_tensor("w_dwn", [5, 128, 9], F32, kind="ExternalInput")
    wdq_d = nc.dram_tensor("w_dwq", [128, 9], F32, kind="ExternalInput")
    wo_d = nc.dram_tensor("w_outT", [C, C], F32, kind="ExternalInput")
    tmp_d = nc.dram_tensor("temp", [C, 1], F32, kind="ExternalInput")
    mask_d = nc.dram_tensor("mask", [C, C], F32, kind="ExternalInput")
    eye8_d = nc.dram_tensor("eye8", [128, 128], F8, kind="ExternalInput")
    eyef_d = nc.dram_tensor("eyef", [128, 128], F32, kind="ExternalInput")
    ones_d = nc.dram_tensor("ones1", [1, 128], F32, kind="ExternalInput")
    if "dummyy" in abl:
        y_d = nc.dram_tensor("y_scratch", [C, N], BF16)
        yprobe_d = nc.dram_tensor("y", [128, 4], BF16, kind="ExternalOutput")
    else:
        y_d = nc.dram_tensor("y", [C, N], BF16, kind="ExternalOutput")
        yprobe_d = None

    with tile.TileContext(nc) as tc, ExitStack() as ctx:
        wp = ctx.enter_context(tc.tile_pool(name="w", bufs=1))
        dwdp = ctx.enter_context(tc.tile_pool(name="dwd", bufs=2))
        sp = ctx.enter_context(tc.tile_pool(name="small", bufs=1))
        yp = ctx.enter_context(tc.tile_pool(name="y", bufs=2))
        gps = ctx.enter_context(tc.tile_pool(name="gps", bufs=2, space="PSUM"))
        dps = ctx.enter_context(tc.tile_pool(name="dps", bufs=2, space="PSUM"))

        # ---- persistent weights ----
        wq0 = wp.tile([128, 576], BF16, tag="wq0")
        wq1 = wp.tile([128, 576], BF16, tag="wq1")  # ch 128..191 duplicated
        nc.sync.dma_start(wq0[:], wq_d[0:128, :])
        nc.sync.dma_start(wq1[0:64, :], wq_d[128:192, :])
        nc.sync.dma_start(wq1[64:128, :], wq_d[128:192, :])
        wo0 = wp.tile([128, C], F32, tag="wo0")
        wo1 = wp.tile([64, C], F32, tag="wo1")
        nc.sync.dma_start(wo0[:], wo_d[0:128, :])
        nc.sync.dma_start(wo1[:], wo_d[128:192, :])
        eye8_s = wp.tile([128, 128], F8, tag="eye8")
        nc.sync.dma_start(eye8_s[:], eye8_d[:])
        eyef_s = wp.tile([128, 128], F32, tag="eyef")
        nc.sync.dma_start(eyef_s[:], eyef_d[:])
        ones_s = wp.tile([1, 128], F32, tag="ones")
        nc.sync.dma_start(ones_s[:], ones_d[:])
        tmp0 = wp.tile([128, 1], F32, tag="tmp0")
        tmp1 = wp.tile([64, 1], F32, tag="tmp1")
        nc.sync.dma_start(tmp0[:], tmp_d[0:128, :])
        nc.sync.dma_start(tmp1[:], tmp_d[128:192, :])
        mask0 = wp.tile([128, C], F32, tag="mask0")
        mask1 = wp.tile([64, C], F32, tag="mask1")
        nc.sync.dma_start(mask0[:], mask_d[0:128, :])
        nc.sync.dma_start(mask1[:], mask_d[128:192, :])
        # M^T (built in phase 1, used in phase 2)
        mt0 = wp.tile([128, C], BF16, tag="mt0")
        mt1 = wp.tile([128, C], BF16, tag="mt1")

        wq8 = wp.tile([128, 2 * 576], F8, tag="wq8")
        nc.sync.dma_start(wq8[:], wq8_d[0])
        wq8b = wp.tile([128, 2 * 576], F8, tag="wq8b")  # ch128.. on rows 64..
        nc.sync.dma_start(wq8b[:], wq8_d[1])
        # resident fp8 x for q,k GEMM: ch 0..127 at 0..N; ch 128..191 folded
        x8 = wp.tile([128, N + N // 2], F8, tag="x8")
        for t0, tw in [(0, 1), (1, 1), (2, 1), (3, 1), (4, 4), (8, 4),
                       (12, 4), (16, 4), (20, 4), (24, 4), (28, 4)]:
            cols = slice(t0 * NT, (t0 + tw) * NT)
            h, lc = t0 // 16, (t0 % 16) * NT
            cols2 = slice(N + lc, N + lc + tw * NT)
            nc.sync.dma_start(x8[:, cols], x8_d[:, cols])
            nc.sync.dma_start(x8[64 * h:64 * h + 64, cols2],
                              x8_d[64 * h:64 * h + 64, cols2])
        # resident bf16 copy of x (v chunks): ch 0..127 at cols 0..N;
        # ch 128..191 folded at cols N..1.5N (spatial half h on parts 64h..)
        xres = wp.tile([128, N + N // 2], BF16, tag="xres")
        for t in range(0, NTILES, 4):
            cols = slice(t * NT, (t + 4) * NT)
            h, lc = t // 16, (t % 16) * NT
            cols1 = slice(N + lc, N + lc + 4 * NT)
            nc.sync.dma_start(xres[:, cols], x_d[0:128, cols])
            nc.sync.dma_start(xres[64 * h:64 * h + 64, cols1],
                              x_d[128:192, cols])

        def gemm_chunk(ci, dst, dst_row0=0):
            """qkv GEMM for chunk ci into dst[dst_row0:dst_row0+mw, :]."""
            mc0, mw = CHUNKS[ci]
            for t in range(NTILES):
                cols = slice(t * NT, (t + 1) * NT)
                h, lc = t // 16, (t % 16) * NT
                cols1 = slice(N + lc, N + lc + NT)
                p64 = slice(64 * h, 64 * h + 64)
                pg = gps.tile([mw, NT], F32, tag="g")
                nc.tensor.matmul(pg[:], wq0[:, mc0:mc0 + mw],
                                 xres[:, cols], start=True, stop=False)
                nc.tensor.matmul(pg[:], wq1[p64, mc0:mc0 + mw],
                                 xres[p64, cols1],
                                 start=False, stop=True,
                                 tile_position=(64 * h, 0))
                if t % 2 == 0:
                    nc.vector.tensor_copy(
                        dst[dst_row0:dst_row0 + mw, cols], pg[:])
                else:
                    nc.scalar.copy(
                        dst[dst_row0:dst_row0 + mw, cols], pg[:])

        def dwconv_v(ci, pre, dwd, dst, dst_row0=0, dwq=None):
            """bf16 depthwise 3x3: diagonal matmuls (PE); every 4th tile on
            DVE as an STT FMA chain to offload the PE."""
            mw = CHUNKS[ci][1]
            for t in range(NTILES):
                base = t * NT
                if dwq is not None and t % 4 == 2:
                    out = dst[dst_row0:dst_row0 + mw, base:base + NT]
                    first = True
                    for ti in TAP_ORDER:
                        dy, dx = TAPS[ti]
                        sh = dy * W + dx
                        lo = max(0, -(base + sh))
                        hi = min(NT, N - base - sh)
                        src_ = pre[0:mw, base + sh + lo:base + sh + hi]
                        wcol = dwq[0:mw, ti:ti + 1]
                        if first:
                            nc.vector.tensor_scalar_mul(out, src_, wcol)
                            first = False
                        else:
                            nc.vector.scalar_tensor_tensor(
                                out=dst[dst_row0:dst_row0 + mw,
                                        base + lo:base + hi],
                                in0=src_, scalar=wcol,
                                in1=dst[dst_row0:dst_row0 + mw,
                                        base + lo:base + hi],
                                op0=ALU.mult, op1=ALU.add)
                    continue
                pd = dps.tile([mw, NT], F32, tag="d")
                for i, ti in enumerate(TAP_ORDER):
                    dy, dx = TAPS[ti]
                    sh = dy * W + dx
                    lo = max(0, -(base + sh))
                    hi = min(NT, N - base - sh)
                    nc.tensor.matmul(
                        pd[:, lo:hi],
                        dwd[0:mw, ti * 128:ti * 128 + mw],
                        pre[0:mw, base + sh + lo:base + sh + hi],
                        start=(i == 0), stop=(i == len(TAP_ORDER) - 1))
                nc.scalar.copy(
                    dst[dst_row0:dst_row0 + mw, base:base + NT], pd[:])

        def edge_fixes(ci, pre, dwn, dst, dst_row0=0):
            """Subtract row-wrap contamination of dx=+-1 taps (strided STT)."""
            mw = CHUNKS[ci][1]
            for dy in (-1, 0, 1):
                ti_l = (dy + 1) * 3 + 0   # (dy, dx=-1)
                y0, y1 = max(0, 1 - dy), min(127, 128 - dy)
                out_ap = dst[dst_row0:dst_row0 + mw, y0 * W:y1 * W + 1:W]
                src_ap = pre[0:mw, (y0 + dy) * W - 1:(y1 + dy) * W:W]
                nc.vector.scalar_tensor_tensor(
                    out=out_ap, in0=src_ap, scalar=dwn[0:mw, ti_l:ti_l + 1],
                    in1=out_ap, op0=ALU.mult, op1=ALU.add)
                ti_r = (dy + 1) * 3 + 2   # (dy, dx=+1)
                y0, y1 = max(0, -1 - dy), min(127, 126 - dy)
                out_ap = dst[dst_row0:dst_row0 + mw,
                             y0 * W + W - 1:y1 * W + W:W]
                src_ap = pre[0:mw, (y0 + dy + 1) * W:(y1 + dy + 1) * W + 1:W]
                nc.vector.scalar_tensor_tensor(
                    out=out_ap, in0=src_ap, scalar=dwn[0:mw, ti_r:ti_r + 1],
                    in1=out_ap, op0=ALU.mult, op1=ALU.add)

        # ================= phase 1: q,k =================
        for _rep in range(reps):
         with tc.tile_pool(name="kT", bufs=1) as kTp, \
             tc.tile_pool(name="qkwork", bufs=1) as qkp, \
             tc.tile_pool(name="junk", bufs=1) as jp, \
             tc.tile_pool(name="qt", bufs=3) as qtp, \
             tc.tile_pool(name="asb", bufs=1) as ap_, \
             tc.tile_pool(name="tps", bufs=2, space="PSUM") as tps, \
             tc.tile_pool(name="aps", bufs=2, space="PSUM") as aps:

            kT = kTp.tile([128, 128 * C], F8, tag="kT")  # block b at b*192
            junk = jp.tile([128, 1024], BF16, tag="junk")
            stat = sp.tile([128, 16], F32, tag="stat")
            apq0 = aps.tile([128, C], F32, tag="attn")
            apq1 = aps.tile([64, C], F32, tag="attn")
            rkrow = sp.tile([1, C], F32, tag="rkrow")

            def k_norms_to_row(rs, plo, phi, koff):
                """rkrow[0, koff:koff+(phi-plo)] = rs[plo:phi] via PE transpose."""
                kw = phi - plo
                pk = tps.tile([1, 128], F32, tag="t")
                nc.tensor.transpose(pk[0:1, 0:kw], rs[plo:phi, 0:1],
                                    eyef_s[plo:phi, plo:plo + kw])
                nc.scalar.copy(rkrow[0:1, koff:koff + kw], pk[0:1, 0:kw])

            kT3 = kT[:].rearrange("p (blk c) -> p blk c", c=C)
            wq83a = wq8[:].rearrange("p (two m) -> p two m", two=2)
            wq83b = wq8b[:].rearrange("p (two m) -> p two m", two=2)

            def process_qk(ci, k_cols=None, q_rows=None):
                """Pipelined q,k chunk in row-padded (pitch P) fp8 layout:
                GEMM tile T -> dw pairs (lag 1) -> transpose/attn (lag 2)."""
                mc0, mw = CHUNKS[ci]
                wdp = dwdp.tile([128, 10 * 128], F8, tag="wdp")
                nc.sync.dma_start(wdp[:], wdp8_d[ci])
                pre8 = qkp.tile([128, NP], F8, tag="pre8", bufs=2)
                nc.any.memset(pre8[:, W::P], 0.0)  # zero row-pad columns
                dw8 = qkp.tile([128, NP], F8, tag="dw8", bufs=1)

                def dw_tile(u):
                    """depthwise for image rows 2u, 2u+1 (NT2 cols)."""
                    base = u * NT2
                    pd = dps.tile([mw, NT2], F32, tag="d")
                    if 1 <= u <= 62:
                        for pi, (tA, tB, off, stride) in enumerate(PAIRS):
                            lhsT = wdp[0:mw, 2 * pi * 128:(2 * pi + 2) * 128]
                            lhsT3 = lhsT.rearrange("q (two m) -> q two m", two=2)
                            nc.tensor.matmul(
                                pd[:], lhsT3[:, :, 0:mw],
                                _pair_ap(pre8, base + off, stride, width=NT2),
                                start=(pi == 0), stop=(pi == len(PAIRS) - 1),
                                perf_mode=DR)
                    else:
                        for i, ti in enumerate(TAP_ORDER):
                            dy, dx = TAPS[ti]
                            sh = dy * P + dx
                            lo = max(0, -(base + sh))
                            hi = min(NT2, NP - base - sh)
                            c0 = STCOL[ti] * 128
                            nc.tensor.matmul(
                                pd[:, lo:hi], wdp[0:mw, c0:c0 + mw],
                                pre8[0:mw, base + sh + lo:base + sh + hi],
                                start=(i == 0), stop=(i == len(TAP_ORDER) - 1))
                    if u % 3 == 0:
                        nc.scalar.copy(dw8[0:mw, base:base + NT2], pd[:])
                    else:
                        nc.vector.tensor_copy(dw8[0:mw, base:base + NT2], pd[:])

                def tr_group(g):
                    """transpose image rows 8g..8g+7, feed kT / attention."""
                    pt = tps.tile([128, 2048], F8, tag="t")
                    for h in range(8):
                        b = 8 * g + h
                        nc.tensor.transpose(
                            _str2_ap(pt, h * 256, mw),
                            dw8[0:mw, b * P:b * P + 128],
                            eye8_s[0:mw, 0:mw])
                    pt3s = _str3_ap(pt, 8, 128)
                    b0 = 8 * g
                    if k_cols is not None:
                        plo, phi, koff = k_cols
                        if g % 2 == 0:
                            nc.vector.tensor_copy(
                                kT3[:, b0:b0 + 8, koff:koff + (phi - plo)],
                                pt3s[:, 0:8, plo:phi])
                        else:
                            nc.scalar.copy(
                                kT3[:, b0:b0 + 8, koff:koff + (phi - plo)],
                                pt3s[:, 0:8, plo:phi])
                    if q_rows is not None:
                        plo, phi, apsum, qw = q_rows
                        qtb = qtp.tile([128, 1024], F8, tag="qt")
                        qtb3 = qtb[:].rearrange("p (h c) -> p h c", c=128)
                        if g % 2 == 0:
                            nc.scalar.copy(qtb3[:, 0:8, 0:qw],
                                           pt3s[:, 0:8, plo:phi])
                        else:
                            nc.vector.tensor_copy(qtb3[:, 0:8, 0:qw],
                                                  pt3s[:, 0:8, plo:phi])
                        for j in (0, 2, 4, 6):
                            kpair = bass.AP(
                                kT[:].tensor, kT[:].offset + (b0 + j) * C,
                                [list(kT[:].ap[0]), [C, 2], [1, C]])
                            nc.tensor.matmul(
                                apsum[0:qw, :], qtb3[:, j:j + 2, 0:qw], kpair,
                                start=(b0 + j == 0), stop=(b0 + j == 126),
                                perf_mode=DR, skip_group_check=True)
                        s = g
                        sq = bass.AP(dw8[:].tensor,
                                     dw8[:].offset + 8 * s * P + plo * 0,
                                     [list(dw8[:].ap[0]), [P, 8], [1, W]])
                        nc.scalar.activation(
                            junk3, sq, AF.Square,
                            accum_out=stat[0:mw, s:s + 1])

                junk3 = junk[0:mw, :].rearrange("p (r c) -> p r c", c=W)
                for T in range(NTILES + 3):
                    if T < NTILES:
                        h, lc = T // 16, (T % 16) * NT
                        wq83 = wq83a if h == 0 else wq83b
                        stride = N + lc - T * NT
                        pg = gps.tile([mw, NT], F32, tag="g")
                        nc.tensor.matmul(pg[:], wq83[:, :, mc0:mc0 + mw],
                                         _pair_ap(x8, T * NT, stride),
                                         start=True, stop=True, perf_mode=DR)
                        dst = bass.AP(pre8[:].tensor, pre8[:].offset + T * 4 * P,
                                      [list(pre8[:].ap[0]), [P, 4], [1, W]])
                        pg3 = pg[:].rearrange("p (r c) -> p r c", c=W)
                        if T % 3 == 0:
                            nc.scalar.copy(dst, pg3)
                        else:
                            nc.vector.tensor_copy(dst, pg3)
                    if 1 <= T <= NTILES:
                        dw_tile(2 * (T - 1))
                        dw_tile(2 * (T - 1) + 1)
                    if T >= 3 and (T - 3) % 2 == 0:
                        tr_group((T - 3) // 2)
                if q_rows is None:
                    return None
                # q-channel norms from the ACT Square accumulations
                mwq = q_rows[1]
                n2 = sp.tile([128, 1], F32, tag="n2")
                nc.vector.tensor_reduce(n2[0:mwq, :], stat[0:mwq, 0:16],
                                        axis=AX.X, op=ALU.add)
                nc.scalar.activation(n2[0:mwq, :], n2[0:mwq, :], AF.Sqrt)
                nc.vector.tensor_scalar_max(n2[0:mwq, :], n2[0:mwq, :], EPS)
                rs = sp.tile([128, 1], F32, tag=f"rs{ci}")
                nc.vector.reciprocal(rs[0:mwq, :], n2[0:mwq, :])
                return rs

            def k_norms_gram(kw, koff, tag):
                """1/||k|| for k-local koff..koff+kw via DR Gram on kT."""
                kg = aps.tile([kw, kw], F32, tag="attn")
                for b in range(0, 128, 2):
                    kp = bass.AP(kT[:].tensor, kT[:].offset + b * C + koff,
                                 [list(kT[:].ap[0]), [C, 2], [1, kw]])
                    nc.tensor.matmul(kg[:], kp, kp, start=(b == 0),
                                     stop=(b == 126), perf_mode=DR,
                                     skip_group_check=True)
                kgd = sp.tile([128, 128], F32, tag="kgd")
                nc.vector.tensor_tensor(kgd[0:kw, 0:kw], kg[:],
                                        eyef_s[0:kw, 0:kw], op=ALU.mult)
                n2 = sp.tile([128, 1], F32, tag="n2")
                nc.vector.tensor_reduce(n2[0:kw, :], kgd[0:kw, 0:kw],
                                        axis=AX.X, op=ALU.add)
                nc.scalar.activation(n2[0:kw, :], n2[0:kw, :], AF.Sqrt)
                nc.vector.tensor_scalar_max(n2[0:kw, :], n2[0:kw, :], EPS)
                rs = sp.tile([128, 1], F32, tag=tag)
                nc.vector.reciprocal(rs[0:kw, :], n2[0:kw, :])
                return rs

            # ---- chunk 2: k channels 256..383 (k-local 64..191) ----
            process_qk(2, k_cols=(0, 128, 64))
            rk2 = k_norms_gram(128, 64, "rk2")
            k_norms_to_row(rk2, 0, 128, 64)

            # ---- chunk 1: q 128..191 (parts 0..63) + k 192..255 (64..127) --
            rs1 = process_qk(1, k_cols=(64, 128, 0), q_rows=(0, 64, apq1, 64))
            rk1 = k_norms_gram(64, 0, "rk1")
            k_norms_to_row(rk1, 0, 64, 0)
            rr1 = sp.tile([64, 1], F32, tag="rr1")
            nc.vector.tensor_tensor(rr1[:], rs1[0:64, :], tmp1[:], op=ALU.mult)

            # ---- chunk 0: q channels 0..127 ----
            rs0 = process_qk(0, q_rows=(0, 128, apq0, 128))
            rr0 = sp.tile([128, 1], F32, tag="rr0")
            nc.vector.tensor_tensor(rr0[:], rs0[:], tmp0[:], op=ALU.mult)

            # ---- 1/||k|| broadcast tile B[p, d] = rkrow[0, d] ----
            bps = tps.tile([128, C], F32, tag="t")
            nc.tensor.matmul(bps[:], ones_s[0:1, 0:128], rkrow[0:1, :],
                             start=True, stop=True)
            bsb = sp.tile([128, C], F32, tag="bsb")
            nc.scalar.copy(bsb[:], bps[:])

            # ---- attention: rescale + mask + softmax + M^T ----
            def softmax_rows(apsum, rr, msk, mw):
                a = ap_.tile([mw, C], F32, tag=f"a{mw}")
                nc.vector.tensor_scalar_mul(a[:], apsum[:], rr[0:mw, :])
                nc.vector.tensor_tensor(a[:], a[:], bsb[0:mw, :], op=ALU.mult)
                nc.vector.tensor_tensor(a[:], a[:], msk[0:mw, :], op=ALU.add)
                mx = sp.tile([128, 1], F32, tag="mx")
                nc.vector.tensor_reduce(mx[0:mw, :], a[:], axis=AX.X, op=ALU.max)
                nmx = sp.tile([128, 1], F32, tag="nmx")
                nc.vector.tensor_scalar_mul(nmx[0:mw, :], mx[0:mw, :], -1.0)
                nc.scalar.activation(a[:], a[:], AF.Exp, bias=nmx[0:mw, :])
                sm = sp.tile([128, 1], F32, tag="sm")
                nc.vector.tensor_reduce(sm[0:mw, :], a[:], axis=AX.X, op=ALU.add)
                rsm = sp.tile([128, 1], F32, tag="rsm")
                nc.vector.reciprocal(rsm[0:mw, :], sm[0:mw, :])
                nc.vector.tensor_scalar_mul(a[:], a[:], rsm[0:mw, :])
                return a
            a0 = softmax_rows(apq0, rr0, mask0, 128)
            a1 = softmax_rows(apq1, rr1, mask1, 64)

            # M^T[d,o] = sum_c A[c,d] w_outT[c,o]; K = c (192 -> 2 chunks)
            for dlo, dw_, mt in ((0, 128, mt0), (128, 64, mt1)):
                pm = tps.tile([128, C], F32, tag="t")
                nc.tensor.matmul(pm[0:dw_, :], a0[:, dlo:dlo + dw_],
                                 wo0[:], start=True, stop=False)
                nc.tensor.matmul(pm[0:dw_, :], a1[:, dlo:dlo + dw_],
                                 wo1[:], start=False, stop=True)
                nc.any.tensor_copy(mt[0:dw_, :], pm[0:dw_, :])
                if dw_ == 64:  # duplicate for folded-v4 tail matmuls
                    nc.any.tensor_copy(mt[64:128, :], pm[0:64, :])

         with tc.tile_pool(name="v3", bufs=1) as v3p, \
             tc.tile_pool(name="v4", bufs=1) as v4p, \
             tc.tile_pool(name="yps", bufs=2, space="PSUM") as yps:
            v3 = v3p.tile([128, N], BF16, tag="v3")
            N2 = N // 2
            v4f = v4p.tile([128, N2], BF16, tag="v4f")  # folded: half h on parts 64h..
            # chunk 3 (v channels 0..127): as before
            pre = v3p.tile([128, N], BF16, tag="pre")
            dwd = dwdp.tile([128, 9 * 128], BF16, tag="dwd")
            nc.sync.dma_start(dwd[:], wdd_d[0])
            dwn = dwdp.tile([128, 9], F32, tag="dwn")
            nc.sync.dma_start(dwn[:], wdn_d[3])
            wdq = dwdp.tile([128, 9], F32, tag="dwn")
            nc.sync.dma_start(wdq[:], wdq_d[:])
            gemm_chunk(3, pre)
            dwconv_v(3, pre, dwd, v3, dwq=wdq)
            edge_fixes(3, pre, dwn, v3)

            # chunk 4 (v channels 128..191) folded into [128, N/2]
            pre4f = v3p.tile([128, N2], BF16, tag="pre")
            dwd4 = dwdp.tile([128, 9 * 128], BF16, tag="dwd")
            nc.sync.dma_start(dwd4[:], wdd_d[1])
            wdb = dwdp.tile([128, 6 * 128], BF16, tag="wdb")
            nc.sync.dma_start(wdb[:], wdb_d[:])
            dwn4 = dwdp.tile([128, 9], F32, tag="dwn")
            nc.sync.dma_start(dwn4[:], wdn_d[4])
            # GEMM into folded layout
            mc0 = CHUNKS[4][0]
            for t in range(NTILES):
                h, lc = t // 16, (t % 16) * NT
                cols = slice(t * NT, (t + 1) * NT)
                cols1 = slice(N + lc, N + lc + NT)
                p64g = slice(64 * h, 64 * h + 64)
                pg = gps.tile([64, NT], F32, tag="g")
                nc.tensor.matmul(pg[:], wq0[:, mc0:mc0 + 64],
                                 xres[:, cols], start=True, stop=False)
                nc.tensor.matmul(pg[:], wq1[p64g, mc0:mc0 + 64],
                                 xres[p64g, cols1], start=False, stop=True,
                                 tile_position=(64 * h, 0))
                if t % 2 == 0:
                    nc.vector.tensor_copy(
                        pre4f[64 * h:64 * h + 64, lc:lc + NT], pg[:])
                else:
                    nc.scalar.copy(
                        pre4f[64 * h:64 * h + 64, lc:lc + NT], pg[:])
            # folded depthwise: 16 tiles over both halves at once
            for t in range(16):
                base = t * NT
                pd = dps.tile([128, NT], F32, tag="d")
                mms = []
                for ti in TAP_ORDER:
                    dy, dx = TAPS[ti]
                    sh = dy * W + dx
                    lo = max(0, -(base + sh))
                    hi = min(NT, N2 - base - sh)
                    mms.append((pd[:, lo:hi],
                                dwd4[0:128, ti * 128:ti * 128 + 128],
                                pre4f[0:128, base + sh + lo:base + sh + hi],
                                None))
                # boundary taps inserted mid-group: the group-closing stop
                # must come from a full-partition matmul
                if t == 15:  # half0 y63 needs image row 64 (half1 parts)
                    for bi, dx in enumerate((-1, 0, 1)):
                        lo, hi = max(0, -dx), min(W, W - dx)
                        mms.insert(1, (pd[0:64, 3 * W + lo:3 * W + hi],
                                       wdb[0:128, bi * 128:bi * 128 + 64],
                                       pre4f[0:128, dx + lo:dx + hi], None))
                if t == 0:   # half1 y0 needs image row 63 (half0 parts)
                    for bi, dx in enumerate((-1, 0, 1)):
                        lo, hi = max(0, -dx), min(W, W - dx)
                        mms.insert(1, (pd[64:128, lo:hi],
                                       wdb[0:128, (3 + bi) * 128 + 64:(3 + bi) * 128 + 128],
                                       pre4f[0:128, 63 * W + dx + lo:63 * W + dx + hi],
                                       (0, 64)))
                for i, (o, l, r, tp) in enumerate(mms):
                    nc.tensor.matmul(o, l, r, start=(i == 0),
                                     stop=(i == len(mms) - 1),
                                     tile_position=tp)
                if t % 3 == 0:
                    nc.scalar.copy(v4f[:, base:base + NT], pd[:])
                else:
                    nc.vector.tensor_copy(v4f[:, base:base + NT], pd[:])
            # folded edge fixes (64-row halves, both halves via partition dim)
            for dy in (-1, 0, 1):
                ti_l = (dy + 1) * 3 + 0
                y0_, y1_ = max(0, 1 - dy), min(63, 64 - dy)
                out_ap = v4f[0:128, y0_ * W:y1_ * W + 1:W]
                src_ap = pre4f[0:128, (y0_ + dy) * W - 1:(y1_ + dy) * W:W]
                nc.vector.scalar_tensor_tensor(
                    out=out_ap, in0=src_ap, scalar=dwn4[0:128, ti_l:ti_l + 1],
                    in1=out_ap, op0=ALU.mult, op1=ALU.add)
                ti_r = (dy + 1) * 3 + 2
                y0_, y1_ = max(0, -1 - dy), min(63, 62 - dy)
                out_ap = v4f[0:128, y0_ * W + W - 1:y1_ * W + W:W]
                src_ap = pre4f[0:128, (y0_ + dy + 1) * W:(y1_ + dy + 1) * W + 1:W]
                nc.vector.scalar_tensor_tensor(
                    out=out_ap, in0=src_ap, scalar=dwn4[0:128, ti_r:ti_r + 1],
                    in1=out_ap, op0=ALU.mult, op1=ALU.add)

            for t in range(NTILES):
                h, lc = t // 16, (t % 16) * NT
                cols = slice(t * NT, (t + 1) * NT)
                lcols = slice(lc, lc + NT)
                p64 = slice(64 * h, 64 * h + 64)
                py0 = yps.tile([128, NT], F32, tag="py0")
                nc.tensor.matmul(py0[:], mt0[:, 0:128], v3[:, cols],
                                 start=True, stop=False)
                nc.tensor.matmul(py0[:], mt1[p64, 0:128], v4f[p64, lcols],
                                 start=False, stop=True,
                                 tile_position=(64 * h, 0))
                y0 = yp.tile([128, NT], BF16, tag="y0", bufs=3)
                if t % 2 == 0:
                    nc.vector.tensor_copy(y0[:], py0[:])
                else:
                    nc.scalar.copy(y0[:], py0[:])
                nc.sync.dma_start(y_d[0:128, cols], y0[:])
                py1 = yps.tile([64, NT], F32, tag="py1")
                nc.tensor.matmul(py1[:], mt0[:, 128:192], v3[:, cols],
                                 start=True, stop=False)
                nc.tensor.matmul(py1[:], mt1[p64, 128:192], v4f[p64, lcols],
                                 start=False, stop=True,
                                 tile_position=(64 * h, 0))
                y1 = yp.tile([64, NT], BF16, tag="y1", bufs=3)
                if t % 2 == 0:
                    nc.scalar.copy(y1[:], py1[:])
                else:
                    nc.vector.tensor_copy(y1[:], py1[:])
                nc.sync.dma_start(y_d[128:192, cols], y1[:])
            if yprobe_d is not None:
                nc.sync.dma_start(yprobe_d[:], y0[:, 0:4])

    nc.compile()
    return nc


def host_inputs(x, w_qkv, w_dw, w_out, temperature):
    """Host-side prep: per-core input maps."""
    b = x.shape[0]
    w_dw9 = np.asarray(w_dw, np.float32).reshape(576, 9)
    # fp8 pair-diag layout for q,k chunks 0..2
    wdp8 = np.zeros((3, 128, 10 * 128), np.float32)
    for ci in range(3):
        s, wid = CHUNKS[ci]
        for p, (tA, tB, _o, _s) in enumerate(PAIRS):
            for h, t in enumerate((tA, tB)):
                if t is None:
                    continue
                c0 = (2 * p + h) * 128
                wdp8[ci, :wid, c0:c0 + wid][np.arange(wid), np.arange(wid)] = \
                    w_dw9[s:s + wid, t]
    # bf16 single-tap diag layout: chunk3 plain; chunk4 folded block-diag2
    wdd = np.zeros((2, 128, 9 * 128), np.float32)
    for t in range(9):
        wdd[0, :128, t * 128:t * 128 + 128][np.arange(128), np.arange(128)] = \
            w_dw9[384:512, t]
        for half in (0, 1):
            o = 64 * half
            wdd[1, o:o + 64, t * 128 + o:t * 128 + o + 64][
                np.arange(64), np.arange(64)] = w_dw9[512:576, t]
    # folded boundary taps: blocks 0..2 dn (dy=+1), 3..5 up (dy=-1)
    wdb = np.zeros((128, 6 * 128), np.float32)
    for bi, t in enumerate((6, 7, 8)):   # (+1,-1),(+1,0),(+1,+1)
        wdb[64 + np.arange(64), bi * 128 + np.arange(64)] = w_dw9[512:576, t]
    for bi, t in enumerate((0, 1, 2)):   # (-1,-1),(-1,0),(-1,+1)
        wdb[np.arange(64), (3 + bi) * 128 + 64 + np.arange(64)] = \
            w_dw9[512:576, t]
    wdn = np.zeros((5, 128, 9), np.float32)
    for ci, (s, wid) in enumerate(CHUNKS):
        wdn[ci, :wid, :] = -w_dw9[s:s + wid, :]
    wdn[4, 64:128, :] = wdn[4, 0:64, :]  # folded: both halves
    wdq = np.ascontiguousarray(w_dw9[384:512, :])  # chunk3 positive taps
    temp_pc = np.repeat(np.asarray(temperature, np.float32).reshape(NHEADS),
                        HDIM).reshape(C, 1)
    mask = np.full((C, C), -1e9, np.float32)
    for h in range(NHEADS):
        mask[h * HDIM:(h + 1) * HDIM, h * HDIM:(h + 1) * HDIM] = 0.0
    wqT = np.ascontiguousarray(np.asarray(w_qkv, np.float32).T)
    wq8 = np.zeros((2, 128, 2, 576), np.float32)
    wq8[:, :, 0, :] = wqT[0:128]
    wq8[0, 0:64, 1, :] = wqT[128:192]
    wq8[1, 64:128, 1, :] = wqT[128:192]
    shared = {
        "w_qkvT": wqT.astype(ml_dtypes.bfloat16),
        "wq8": wq8.reshape(2, 128, 2 * 576).astype(ml_dtypes.float8_e4m3),
        "w_dwb": wdb.astype(ml_dtypes.bfloat16),
        "w_dwp8": wdp8.astype(ml_dtypes.float8_e4m3),
        "w_dwd": wdd.astype(ml_dtypes.bfloat16),
        "w_dwn": wdn,
        "w_dwq": wdq,
        "w_outT": np.ascontiguousarray(np.asarray(w_out, np.float32).T),
        "temp": temp_pc,
        "mask": mask,
        "eye8": np.eye(128, dtype=ml_dtypes.float8_e4m3),
        "eyef": np.eye(128, dtype=np.float32),
        "ones1": np.ones((1, 128), np.float32),
    }
    out_maps = []
    for c in range(b):
        xc = np.ascontiguousarray(np.asarray(x[c], np.float32).reshape(C, N))
        x8 = np.zeros((128, N + N // 2), np.float32)
        x8[:, 0:N] = xc[0:128]
        x8[0:64, N:N + N // 2] = xc[128:192, 0:N // 2]
        x8[64:128, N:N + N // 2] = xc[128:192, N // 2:N]
        out_maps.append(dict(shared, x=xc.astype(ml_dtypes.bfloat16),
                             x8=x8.astype(ml_dtypes.float8_e4m3)))
    return out_maps


_NC_CACHE = {}


def kernel(x, w_qkv, w_dw, w_out, temperature):
    x = np.asarray(x)
    if "nc" not in _NC_CACHE:
        _NC_CACHE["nc"] = build_nc()
    nc = _NC_CACHE["nc"]
    in_maps = host_inputs(x, w_qkv, w_dw, w_out, temperature)
    res = run_bass_kernel_spmd(nc, in_maps, list(range(8)))
    out = np.stack([np.asarray(res.results[c]["y"]).astype(np.float32)
                    .reshape(C, H, W) for c in range(8)])
    return out


# revision 28
# speedup vs baseline: 5.5312x; 2.2869x over previous
"""MDTA (Restormer transposed attention) Trainium2 Bass kernel.

Data-parallel over batch (8 cores x 1 image). Per core:
  1. qkv 1x1 conv as bf16 GEMM (PE); q,k chunk outputs stored fp8e4,
     v chunks bf16.
  2. depthwise 3x3 conv: q,k chunks via fp8 DoubleRow diagonal matmuls
     (2 taps per matmul, strided pair APs, 2x PE throughput); interior
     tiles use 5 pair-matmuls, edge tiles 9 singles with AP clipping.
     v chunks via bf16 diagonal matmuls. Row-wrap contamination of
     dx=+-1 taps fixed with strided DVE STT ops.
  3. q,k kept UNNORMALIZED in fp8: PE-transpose (stride-2 fp8 out) to
     spatial-part layout, Gram G = q^T k via fp8 DoubleRow over block
     pairs. Normalization applied to the 192x192 logits instead:
     row scale temp/||q|| (per-partition) and column scale 1/||k||
     (broadcast tile built by a K=1 matmul from PE-transposed norms).
  4. blockdiag mask + softmax -> A; M^T = A^T w_out^T; y = M^T.T @ v.
"""
import sys
sys.path.insert(0, "/opt/trn_rl_repo")
from contextlib import ExitStack

import numpy as np
import ml_dtypes

import concourse.bass as bass
import concourse.mybir as mybir
import concourse.tile as tile
from concourse import bacc
from concourse.bass_utils import run_bass_kernel_spmd

F32 = mybir.dt.float32
BF16 = mybir.dt.bfloat16
F8 = mybir.dt.float8e4
DR = mybir.MatmulPerfMode.DoubleRow
AF = mybir.ActivationFunctionType
ALU = mybir.AluOpType
AX = mybir.AxisListType

C = 192
NHEADS = 8
HDIM = 24
H = W = 128
N = H * W            # 16384 spatial positions
NT = 512             # free-dim tile (4 image rows)
NTILES = N // NT     # 32
P = W + 1            # padded row pitch for q,k chunks (zero col kills wrap)
NP = H * P           # padded row-major size
NT2 = 2 * P          # dw tile: 2 image rows (psum bank fits 258 f32)
EPS = 1e-12
# output-channel chunks of the 576-row qkv: q = 0..191, k = 192..383,
# v = 384..575
CHUNKS = [(0, 128), (128, 128), (256, 128), (384, 128), (512, 64)]
TAPS = [(dy, dx) for dy in (-1, 0, 1) for dx in (-1, 0, 1)]
TAP_ORDER = [4, 0, 1, 2, 3, 5, 6, 7, 8]  # center tap first (start=True)
# fp8 tap pairs: (tapA, tapB, rhs offset, rhs stride); tapB None => zero
PAIRS = [(4, None, 0, 1),          # center first: start=True
         (0, 2, -P - 1, 2),       # (-1,-1) + (-1,+1)
         (3, 5, -1, 2),           # (0,-1)  + (0,+1)
         (6, 8, P - 1, 2),        # (+1,-1) + (+1,+1)
         (1, 7, -P, 2 * P)]       # (-1,0)  + (+1,0)
# single-tap column (x128) in the wdp8 pair layout
STCOL = {}
for _p, (_a, _b, _o, _s) in enumerate(PAIRS):
    STCOL[_a] = 2 * _p
    if _b is not None:
        STCOL[_b] = 2 * _p + 1


def _pair_ap(t, off, stride, width=NT):
    """rhs AP [parts][2, stride][width, 1] at element offset off."""
    a = t[:]
    return bass.AP(a.tensor, a.offset + off,
                   [list(a.ap[0]), [stride, 2], [1, width]])


def _str2_ap(t, off, n):
    """fp8 transpose out AP: [parts][n, step 2] at element offset off."""
    a = t[:]
    return bass.AP(a.tensor, a.offset + off, [list(a.ap[0]), [2, n]])


def _str3_ap(t, npair, m):
    """read-back AP over stride-2 transpose pairs: [parts][npair, 256][m, 2]."""
    a = t[:]
    return bass.AP(a.tensor, a.offset, [list(a.ap[0]), [256, npair], [2, m]])


def build_nc(reps=1, abl=()):  # noqa: C901
    nc = bacc.Bacc("TRN2", target_bir_lowering=False, debug=False)
    x_d = (nc.dram_tensor("x_scratch", [C, N], BF16) if "dummyx" in abl
           else nc.dram_tensor("x", [C, N], BF16, kind="ExternalInput"))
    wq_d = nc.dram_tensor("w_qkvT", [C, 576], BF16, kind="ExternalInput")
    x8_d = (nc.dram_tensor("x8_scratch", [128, N + N // 2], F8) if "dummyx"
            in abl else nc.dram_tensor("x8", [128, N + N // 2], F8,
                                       kind="ExternalInput"))
    wq8_d = nc.dram_tensor("wq8", [2, 128, 2 * 576], F8, kind="ExternalInput")
    wdp8_d = nc.dram_tensor("w_dwp8", [3, 128, 10 * 128], F8, kind="ExternalInput")
    wdd_d = nc.dram_tensor("w_dwd", [2, 128, 9 * 128], BF16, kind="ExternalInput")
    wdb_d = nc.dram_tensor("w_dwb", [128, 6 * 128], BF16, kind="ExternalInput")
    wdn_d = nc.dram_tensor("w_dwn", [5, 128, 9], F32, kind="ExternalInput")
    wdq_d = nc.dram_tensor("w_dwq", [128, 9], F32, kind="ExternalInput")
    wo_d = nc.dram_tensor("w_outT", [C, C], F32, kind="ExternalInput")
    tmp_d = nc.dram_tensor("temp", [C, 1], F32, kind="ExternalInput")
    mask_d = nc.dram_tensor("mask", [C, C], F32, kind="ExternalInput")
    eye8_d = nc.dram_tensor("eye8", [128, 128], F8, kind="ExternalInput")
    eyef_d = nc.dram_tensor("eyef", [128, 128], F32, kind="ExternalInput")
    ones_d = nc.dram_tensor("ones1", [1, 128], F32, kind="ExternalInput")
    if "dummyy" in abl:
        y_d = nc.dram_tensor("y_scratch", [C, N], BF16)
        yprobe_d = nc.dram_tensor("y", [128, 4], BF16, kind="ExternalOutput")
    else:
        y_d = nc.dram_tensor("y", [C, N], BF16, kind="ExternalOutput")
        yprobe_d = None

    with tile.TileContext(nc) as tc, ExitStack() as ctx:
        wp = ctx.enter_context(tc.tile_pool(name="w", bufs=1))
        dwdp = ctx.enter_context(tc.tile_pool(name="dwd", bufs=2))
        sp = ctx.enter_context(tc.tile_pool(name="small", bufs=1))
        yp = ctx.enter_context(tc.tile_pool(name="y", bufs=2))
        gps = ctx.enter_context(tc.tile_pool(name="gps", bufs=2, space="PSUM"))
        dps = ctx.enter_context(tc.tile_pool(name="dps", bufs=2, space="PSUM"))

        # ---- fp8 GEMM operands first: phase 1 starts as soon as these land
        wq8 = wp.tile([128, 2 * 576], F8, tag="wq8")
        nc.sync.dma_start(wq8[:], wq8_d[0])
        wq8b = wp.tile([128, 2 * 576], F8, tag="wq8b")  # ch128.. on rows 64..
        nc.sync.dma_start(wq8b[:], wq8_d[1])
        wdp2 = None  # chunk2 dw weights: loaded before x so phase 1 starts hot
        # resident fp8 x for q,k GEMM: ch 0..127 at 0..N; ch 128..191 folded
        x8 = wp.tile([128, N + N // 2], F8, tag="x8")
        wdp2 = dwdp.tile([128, 10 * 128], F8, tag="wdp")
        nc.sync.dma_start(wdp2[:], wdp8_d[2])
        for t0, tw in [(0, 1), (1, 1), (2, 1), (3, 1), (4, 4), (8, 4),
                       (12, 4), (16, 4), (20, 4), (24, 4), (28, 4)]:
            cols = slice(t0 * NT, (t0 + tw) * NT)
            h, lc = t0 // 16, (t0 % 16) * NT
            cols2 = slice(N + lc, N + lc + tw * NT)
            nc.sync.dma_start(x8[:, cols], x8_d[:, cols])
            nc.sync.dma_start(x8[64 * h:64 * h + 64, cols2],
                              x8_d[64 * h:64 * h + 64, cols2])
        # ---- persistent weights ----
        wq0 = wp.tile([128, 576], BF16, tag="wq0")
        wq1 = wp.tile([128, 576], BF16, tag="wq1")  # ch 128..191 duplicated
        nc.sync.dma_start(wq0[:], wq_d[0:128, :])
        nc.sync.dma_start(wq1[0:64, :], wq_d[128:192, :])
        nc.sync.dma_start(wq1[64:128, :], wq_d[128:192, :])
        wo0 = wp.tile([128, C], F32, tag="wo0")
        wo1 = wp.tile([64, C], F32, tag="wo1")
        nc.sync.dma_start(wo0[:], wo_d[0:128, :])
        nc.sync.dma_start(wo1[:], wo_d[128:192, :])
        eye8_s = wp.tile([128, 128], F8, tag="eye8")
        nc.sync.dma_start(eye8_s[:], eye8_d[:])
        eyef_s = wp.tile([128, 128], F32, tag="eyef")
        nc.sync.dma_start(eyef_s[:], eyef_d[:])
        ones_s = wp.tile([1, 128], F32, tag="ones")
        nc.sync.dma_start(ones_s[:], ones_d[:])
        tmp0 = wp.tile([128, 1], F32, tag="tmp0")
        tmp1 = wp.tile([64, 1], F32, tag="tmp1")
        nc.sync.dma_start(tmp0[:], tmp_d[0:128, :])
        nc.sync.dma_start(tmp1[:], tmp_d[128:192, :])
        mask0 = wp.tile([128, C], F32, tag="mask0")
        mask1 = wp.tile([64, C], F32, tag="mask1")
        nc.sync.dma_start(mask0[:], mask_d[0:128, :])
        nc.sync.dma_start(mask1[:], mask_d[128:192, :])
        # M^T (built in phase 1, used in phase 2)
        mt0 = wp.tile([128, C], BF16, tag="mt0")
        mt1 = wp.tile([128, C], BF16, tag="mt1")

        # resident bf16 copy of x (v chunks): ch 0..127 at cols 0..N;
        # ch 128..191 folded at cols N..1.5N (spatial half h on parts 64h..)
        xres = wp.tile([128, N + N // 2], BF16, tag="xres")
        for t in range(0, NTILES, 4):
            cols = slice(t * NT, (t + 4) * NT)
            h, lc = t // 16, (t % 16) * NT
            cols1 = slice(N + lc, N + lc + 4 * NT)
            nc.sync.dma_start(xres[:, cols], x_d[0:128, cols])
            nc.sync.dma_start(xres[64 * h:64 * h + 64, cols1],
                              x_d[128:192, cols])

        def gemm_chunk(ci, dst, dst_row0=0):
            """qkv GEMM for chunk ci into dst[dst_row0:dst_row0+mw, :]."""
            mc0, mw = CHUNKS[ci]
            for t in range(NTILES):
                cols = slice(t * NT, (t + 1) * NT)
                h, lc = t // 16, (t % 16) * NT
                cols1 = slice(N + lc, N + lc + NT)
                p64 = slice(64 * h, 64 * h + 64)
                pg = gps.tile([mw, NT], F32, tag="g")
                nc.tensor.matmul(pg[:], wq0[:, mc0:mc0 + mw],
                                 xres[:, cols], start=True, stop=False)
                nc.tensor.matmul(pg[:], wq1[p64, mc0:mc0 + mw],
                                 xres[p64, cols1],
                                 start=False, stop=True,
                                 tile_position=(64 * h, 0))
                if t % 2 == 0:
                    nc.vector.tensor_copy(
                        dst[dst_row0:dst_row0 + mw, cols], pg[:])
                else:
                    nc.scalar.copy(
                        dst[dst_row0:dst_row0 + mw, cols], pg[:])

        def dwconv_v(ci, pre, dwd, dst, dst_row0=0, dwq=None):
            """bf16 depthwise 3x3: diagonal matmuls (PE); every 4th tile on
            DVE as an STT FMA chain to offload the PE."""
            mw = CHUNKS[ci][1]
            for t in range(NTILES):
                base = t * NT
                if dwq is not None and t % 4 == 2:
                    out = dst[dst_row0:dst_row0 + mw, base:base + NT]
                    first = True
                    for ti in TAP_ORDER:
                        dy, dx = TAPS[ti]
                        sh = dy * W + dx
                        lo = max(0, -(base + sh))
                        hi = min(NT, N - base - sh)
                        src_ = pre[0:mw, base + sh + lo:base + sh + hi]
                        wcol = dwq[0:mw, ti:ti + 1]
                        if first:
                            nc.vector.tensor_scalar_mul(out, src_, wcol)
                            first = False
                        else:
                            nc.vector.scalar_tensor_tensor(
                                out=dst[dst_row0:dst_row0 + mw,
                                        base + lo:base + hi],
                                in0=src_, scalar=wcol,
                                in1=dst[dst_row0:dst_row0 + mw,
                                        base + lo:base + hi],
                                op0=ALU.mult, op1=ALU.add)
                    continue
                pd = dps.tile([mw, NT], F32, tag="d")
                for i, ti in enumerate(TAP_ORDER):
                    dy, dx = TAPS[ti]
                    sh = dy * W + dx
                    lo = max(0, -(base + sh))
                    hi = min(NT, N - base - sh)
                    nc.tensor.matmul(
                        pd[:, lo:hi],
                        dwd[0:mw, ti * 128:ti * 128 + mw],
                        pre[0:mw, base + sh + lo:base + sh + hi],
                        start=(i == 0), stop=(i == len(TAP_ORDER) - 1))
                nc.scalar.copy(
                    dst[dst_row0:dst_row0 + mw, base:base + NT], pd[:])

        def edge_fixes(ci, pre, dwn, dst, dst_row0=0):
            """Subtract row-wrap contamination of dx=+-1 taps (strided STT)."""
            mw = CHUNKS[ci][1]
            for dy in (-1, 0, 1):
                ti_l = (dy + 1) * 3 + 0   # (dy, dx=-1)
                y0, y1 = max(0, 1 - dy), min(127, 128 - dy)
                out_ap = dst[dst_row0:dst_row0 + mw, y0 * W:y1 * W + 1:W]
                src_ap = pre[0:mw, (y0 + dy) * W - 1:(y1 + dy) * W:W]
                nc.vector.scalar_tensor_tensor(
                    out=out_ap, in0=src_ap, scalar=dwn[0:mw, ti_l:ti_l + 1],
                    in1=out_ap, op0=ALU.mult, op1=ALU.add)
                ti_r = (dy + 1) * 3 + 2   # (dy, dx=+1)
                y0, y1 = max(0, -1 - dy), min(127, 126 - dy)
                out_ap = dst[dst_row0:dst_row0 + mw,
                             y0 * W + W - 1:y1 * W + W:W]
                src_ap = pre[0:mw, (y0 + dy + 1) * W:(y1 + dy + 1) * W + 1:W]
                nc.vector.scalar_tensor_tensor(
                    out=out_ap, in0=src_ap, scalar=dwn[0:mw, ti_r:ti_r + 1],
                    in1=out_ap, op0=ALU.mult, op1=ALU.add)

        # ================= phase 1: q,k =================
        for _rep in range(reps):
         with tc.tile_pool(name="kT", bufs=1) as kTp, \
             tc.tile_pool(name="qkwork", bufs=1) as qkp, \
             tc.tile_pool(name="junk", bufs=1) as jp, \
             tc.tile_pool(name="qt", bufs=3) as qtp, \
             tc.tile_pool(name="asb", bufs=1) as ap_, \
             tc.tile_pool(name="tps", bufs=2, space="PSUM") as tps, \
             tc.tile_pool(name="aps", bufs=2, space="PSUM") as aps:

            kT = kTp.tile([128, 128 * C], F8, tag="kT")  # block b at b*192
            junk = jp.tile([128, 1024], BF16, tag="junk")
            stat = sp.tile([128, 16], F32, tag="stat")
            apq0 = aps.tile([128, C], F32, tag="attn")
            apq1 = aps.tile([64, C], F32, tag="attn")
            rkrow = sp.tile([1, C], F32, tag="rkrow")

            def k_norms_to_row(rs, plo, phi, koff):
                """rkrow[0, koff:koff+(phi-plo)] = rs[plo:phi] via PE transpose."""
                kw = phi - plo
                pk = tps.tile([1, 128], F32, tag="t")
                nc.tensor.transpose(pk[0:1, 0:kw], rs[plo:phi, 0:1],
                                    eyef_s[plo:phi, plo:plo + kw])
                nc.scalar.copy(rkrow[0:1, koff:koff + kw], pk[0:1, 0:kw])

            kT3 = kT[:].rearrange("p (blk c) -> p blk c", c=C)
            wq83a = wq8[:].rearrange("p (two m) -> p two m", two=2)
            wq83b = wq8b[:].rearrange("p (two m) -> p two m", two=2)

            def process_qk(ci, k_cols=None, q_rows=None, wdp=None,
                           amod=3):
                """Pipelined q,k chunk in row-padded (pitch P) fp8 layout:
                GEMM tile T -> dw pairs (lag 1) -> transpose/attn (lag 2)."""
                mc0, mw = CHUNKS[ci]
                if wdp is None:
                    wdp = dwdp.tile([128, 10 * 128], F8, tag="wdp")
                    nc.sync.dma_start(wdp[:], wdp8_d[ci])
                pre8 = qkp.tile([128, NP], F8, tag="pre8", bufs=2)
                nc.any.memset(pre8[:, W::P], 0.0)  # zero row-pad columns
                dw8 = qkp.tile([128, NP], F8, tag="dw8", bufs=2)

                def dw_tile(u):
                    """depthwise for image rows 2u, 2u+1 (NT2 cols)."""
                    base = u * NT2
                    pd = dps.tile([mw, NT2], F32, tag="d")
                    if 1 <= u <= 62:
                        for pi, (tA, tB, off, stride) in enumerate(PAIRS):
                            lhsT = wdp[0:mw, 2 * pi * 128:(2 * pi + 2) * 128]
                            lhsT3 = lhsT.rearrange("q (two m) -> q two m", two=2)
                            nc.tensor.matmul(
                                pd[:], lhsT3[:, :, 0:mw],
                                _pair_ap(pre8, base + off, stride, width=NT2),
                                start=(pi == 0), stop=(pi == len(PAIRS) - 1),
                                perf_mode=DR)
                    else:
                        for i, ti in enumerate(TAP_ORDER):
                            dy, dx = TAPS[ti]
                            sh = dy * P + dx
                            lo = max(0, -(base + sh))
                            hi = min(NT2, NP - base - sh)
                            c0 = STCOL[ti] * 128
                            nc.tensor.matmul(
                                pd[:, lo:hi], wdp[0:mw, c0:c0 + mw],
                                pre8[0:mw, base + sh + lo:base + sh + hi],
                                start=(i == 0), stop=(i == len(TAP_ORDER) - 1))
                    if u % amod == 0:
                        nc.scalar.copy(dw8[0:mw, base:base + NT2], pd[:])
                    else:
                        nc.vector.tensor_copy(dw8[0:mw, base:base + NT2], pd[:])

                def tr_group(g):
                    """transpose image rows 8g..8g+7, feed kT / attention."""
                    pt = tps.tile([128, 2048], F8, tag="t")
                    for h in range(8):
                        b = 8 * g + h
                        nc.tensor.transpose(
                            _str2_ap(pt, h * 256, mw),
                            dw8[0:mw, b * P:b * P + 128],
                            eye8_s[0:mw, 0:mw])
                    pt3s = _str3_ap(pt, 8, 128)
                    b0 = 8 * g
                    if k_cols is not None:
                        plo, phi, koff = k_cols
                        if g % 2 == 0:
                            nc.vector.tensor_copy(
                                kT3[:, b0:b0 + 8, koff:koff + (phi - plo)],
                                pt3s[:, 0:8, plo:phi])
                        else:
                            nc.scalar.copy(
                                kT3[:, b0:b0 + 8, koff:koff + (phi - plo)],
                                pt3s[:, 0:8, plo:phi])
                    if q_rows is not None:
                        plo, phi, apsum, qw = q_rows
                        qtb = qtp.tile([128, 1024], F8, tag="qt")
                        qtb3 = qtb[:].rearrange("p (h c) -> p h c", c=128)
                        if g % 2 == 0:
                            nc.scalar.copy(qtb3[:, 0:8, 0:qw],
                                           pt3s[:, 0:8, plo:phi])
                        else:
                            nc.vector.tensor_copy(qtb3[:, 0:8, 0:qw],
                                                  pt3s[:, 0:8, plo:phi])
                        for j in (0, 2, 4, 6):
                            kpair = bass.AP(
                                kT[:].tensor, kT[:].offset + (b0 + j) * C,
                                [list(kT[:].ap[0]), [C, 2], [1, C]])
                            nc.tensor.matmul(
                                apsum[0:qw, :], qtb3[:, j:j + 2, 0:qw], kpair,
                                start=(b0 + j == 0), stop=(b0 + j == 126),
                                perf_mode=DR, skip_group_check=True)
                        s = g
                        sq = bass.AP(dw8[:].tensor,
                                     dw8[:].offset + 8 * s * P + plo * 0,
                                     [list(dw8[:].ap[0]), [P, 8], [1, W]])
                        nc.scalar.activation(
                            junk3, sq, AF.Square,
                            accum_out=stat[0:mw, s:s + 1])

                junk3 = junk[0:mw, :].rearrange("p (r c) -> p r c", c=W)
                for T in range(NTILES + 3):
                    if T < NTILES:
                        h, lc = T // 16, (T % 16) * NT
                        wq83 = wq83a if h == 0 else wq83b
                        stride = N + lc - T * NT
                        pg = gps.tile([mw, NT], F32, tag="g")
                        nc.tensor.matmul(pg[:], wq83[:, :, mc0:mc0 + mw],
                                         _pair_ap(x8, T * NT, stride),
                                         start=True, stop=True, perf_mode=DR)
                        dst = bass.AP(pre8[:].tensor, pre8[:].offset + T * 4 * P,
                                      [list(pre8[:].ap[0]), [P, 4], [1, W]])
                        pg3 = pg[:].rearrange("p (r c) -> p r c", c=W)
                        if T % amod == 0:
                            nc.scalar.copy(dst, pg3)
                        else:
                            nc.vector.tensor_copy(dst, pg3)
                    if 1 <= T <= NTILES:
                        dw_tile(2 * (T - 1))
                        dw_tile(2 * (T - 1) + 1)
                    if T >= 3 and (T - 3) % 2 == 0:
                        tr_group((T - 3) // 2)
                if q_rows is None:
                    return None
                # q-channel norms from the ACT Square accumulations
                mwq = q_rows[1]
                n2 = sp.tile([128, 1], F32, tag="n2")
                nc.vector.tensor_reduce(n2[0:mwq, :], stat[0:mwq, 0:16],
                                        axis=AX.X, op=ALU.add)
                nc.scalar.activation(n2[0:mwq, :], n2[0:mwq, :], AF.Sqrt)
                nc.vector.tensor_scalar_max(n2[0:mwq, :], n2[0:mwq, :], EPS)
                rs = sp.tile([128, 1], F32, tag=f"rs{ci}")
                nc.vector.reciprocal(rs[0:mwq, :], n2[0:mwq, :])
                return rs

            def k_norms_gram(kw, koff, tag):
                """1/||k|| for k-local koff..koff+kw via DR Gram on kT."""
                kg = aps.tile([kw, kw], F32, tag="attn")
                for b in range(0, 128, 2):
                    kp = bass.AP(kT[:].tensor, kT[:].offset + b * C + koff,
                                 [list(kT[:].ap[0]), [C, 2], [1, kw]])
                    nc.tensor.matmul(kg[:], kp, kp, start=(b == 0),
                                     stop=(b == 126), perf_mode=DR,
                                     skip_group_check=True)
                kgd = sp.tile([128, 128], F32, tag="kgd")
                nc.vector.tensor_tensor(kgd[0:kw, 0:kw], kg[:],
                                        eyef_s[0:kw, 0:kw], op=ALU.mult)
                n2 = sp.tile([128, 1], F32, tag="n2")
                nc.vector.tensor_reduce(n2[0:kw, :], kgd[0:kw, 0:kw],
                                        axis=AX.X, op=ALU.add)
                nc.scalar.activation(n2[0:kw, :], n2[0:kw, :], AF.Sqrt)
                nc.vector.tensor_scalar_max(n2[0:kw, :], n2[0:kw, :], EPS)
                rs = sp.tile([128, 1], F32, tag=tag)
                nc.vector.reciprocal(rs[0:kw, :], n2[0:kw, :])
                return rs

            # ---- chunk 2: k channels 256..383 (k-local 64..191) ----
            process_qk(2, k_cols=(0, 128, 64), wdp=wdp2, amod=2)
            rk2 = k_norms_gram(128, 64, "rk2")
            k_norms_to_row(rk2, 0, 128, 64)

            # ---- chunk 1: q 128..191 (parts 0..63) + k 192..255 (64..127) --
            rs1 = process_qk(1, k_cols=(64, 128, 0), q_rows=(0, 64, apq1, 64))
            rk1 = k_norms_gram(64, 0, "rk1")
            k_norms_to_row(rk1, 0, 64, 0)
            rr1 = sp.tile([64, 1], F32, tag="rr1")
            nc.vector.tensor_tensor(rr1[:], rs1[0:64, :], tmp1[:], op=ALU.mult)

            # ---- chunk 0: q channels 0..127 ----
            rs0 = process_qk(0, q_rows=(0, 128, apq0, 128))
            rr0 = sp.tile([128, 1], F32, tag="rr0")
            nc.vector.tensor_tensor(rr0[:], rs0[:], tmp0[:], op=ALU.mult)

            # ---- 1/||k|| broadcast tile B[p, d] = rkrow[0, d] ----
            bps = tps.tile([128, C], F32, tag="t")
            nc.tensor.matmul(bps[:], ones_s[0:1, 0:128], rkrow[0:1, :],
                             start=True, stop=True)
            bsb = sp.tile([128, C], F32, tag="bsb")
            nc.scalar.copy(bsb[:], bps[:])

            # ---- attention: rescale + mask + softmax + M^T ----
            def softmax_rows(apsum, rr, msk, mw):
                a = ap_.tile([mw, C], F32, tag=f"a{mw}")
                nc.vector.tensor_scalar_mul(a[:], apsum[:], rr[0:mw, :])
                nc.vector.tensor_tensor(a[:], a[:], bsb[0:mw, :], op=ALU.mult)
                nc.vector.tensor_tensor(a[:], a[:], msk[0:mw, :], op=ALU.add)
                mx = sp.tile([128, 1], F32, tag="mx")
                nc.vector.tensor_reduce(mx[0:mw, :], a[:], axis=AX.X, op=ALU.max)
                nmx = sp.tile([128, 1], F32, tag="nmx")
                nc.vector.tensor_scalar_mul(nmx[0:mw, :], mx[0:mw, :], -1.0)
                nc.scalar.activation(a[:], a[:], AF.Exp, bias=nmx[0:mw, :])
                sm = sp.tile([128, 1], F32, tag="sm")
                nc.vector.tensor_reduce(sm[0:mw, :], a[:], axis=AX.X, op=ALU.add)
                rsm = sp.tile([128, 1], F32, tag="rsm")
                nc.vector.reciprocal(rsm[0:mw, :], sm[0:mw, :])
                nc.vector.tensor_scalar_mul(a[:], a[:], rsm[0:mw, :])
                return a
            a0 = softmax_rows(apq0, rr0, mask0, 128)
            a1 = softmax_rows(apq1, rr1, mask1, 64)

            # M^T[d,o] = sum_c A[c,d] w_outT[c,o]; K = c (192 -> 2 chunks)
            for dlo, dw_, mt in ((0, 128, mt0), (128, 64, mt1)):
                pm = tps.tile([128, C], F32, tag="t")
                nc.tensor.matmul(pm[0:dw_, :], a0[:, dlo:dlo + dw_],
                                 wo0[:], start=True, stop=False)
                nc.tensor.matmul(pm[0:dw_, :], a1[:, dlo:dlo + dw_],
                                 wo1[:], start=False, stop=True)
                nc.any.tensor_copy(mt[0:dw_, :], pm[0:dw_, :])
                if dw_ == 64:  # duplicate for folded-v4 tail matmuls
                    nc.any.tensor_copy(mt[64:128, :], pm[0:64, :])

         with tc.tile_pool(name="v3", bufs=1) as v3p, \
             tc.tile_pool(name="v4", bufs=1) as v4p, \
             tc.tile_pool(name="yps", bufs=2, space="PSUM") as yps:
            v3 = v3p.tile([128, N], BF16, tag="v3")
            N2 = N // 2
            v4f = v4p.tile([128, N2], BF16, tag="v4f")  # folded: half h on parts 64h..
            # chunk 3 (v channels 0..127): as before
            pre = v3p.tile([128, N], BF16, tag="pre")
            dwd = dwdp.tile([128, 9 * 128], BF16, tag="dwd")
            nc.sync.dma_start(dwd[:], wdd_d[0])
            dwn = dwdp.tile([128, 9], F32, tag="dwn")
            nc.sync.dma_start(dwn[:], wdn_d[3])
            wdq = dwdp.tile([128, 9], F32, tag="dwn")
            nc.sync.dma_start(wdq[:], wdq_d[:])
            gemm_chunk(3, pre)
            dwconv_v(3, pre, dwd, v3, dwq=wdq)
            edge_fixes(3, pre, dwn, v3)

            # chunk 4 (v channels 128..191) folded into [128, N/2]
            pre4f = v3p.tile([128, N2], BF16, tag="pre")
            dwd4 = dwdp.tile([128, 9 * 128], BF16, tag="dwd")
            nc.sync.dma_start(dwd4[:], wdd_d[1])
            wdb = dwdp.tile([128, 6 * 128], BF16, tag="wdb")
            nc.sync.dma_start(wdb[:], wdb_d[:])
            dwn4 = dwdp.tile([128, 9], F32, tag="dwn")
            nc.sync.dma_start(dwn4[:], wdn_d[4])
            # GEMM into folded layout
            mc0 = CHUNKS[4][0]
            for t in range(NTILES):
                h, lc = t // 16, (t % 16) * NT
                cols = slice(t * NT, (t + 1) * NT)
                cols1 = slice(N + lc, N + lc + NT)
                p64g = slice(64 * h, 64 * h + 64)
                pg = gps.tile([64, NT], F32, tag="g")
                nc.tensor.matmul(pg[:], wq0[:, mc0:mc0 + 64],
                                 xres[:, cols], start=True, stop=False)
                nc.tensor.matmul(pg[:], wq1[p64g, mc0:mc0 + 64],
                                 xres[p64g, cols1], start=False, stop=True,
                                 tile_position=(64 * h, 0))
                if t % 2 == 0:
                    nc.vector.tensor_copy(
                        pre4f[64 * h:64 * h + 64, lc:lc + NT], pg[:])
                else:
                    nc.scalar.copy(
                        pre4f[64 * h:64 * h + 64, lc:lc + NT], pg[:])
            # folded depthwise: 16 tiles over both halves at once
            for t in range(16):
                base = t * NT
                pd = dps.tile([128, NT], F32, tag="d")
                mms = []
                for ti in TAP_ORDER:
                    dy, dx = TAPS[ti]
                    sh = dy * W + dx
                    lo = max(0, -(base + sh))
                    hi = min(NT, N2 - base - sh)
                    mms.append((pd[:, lo:hi],
                                dwd4[0:128, ti * 128:ti * 128 + 128],
                                pre4f[0:128, base + sh + lo:base + sh + hi],
                                None))
                # boundary taps inserted mid-group: the group-closing stop
                # must come from a full-partition matmul
                if t == 15:  # half0 y63 needs image row 64 (half1 parts)
                    for bi, dx in enumerate((-1, 0, 1)):
                        lo, hi = max(0, -dx), min(W, W - dx)
                        mms.insert(1, (pd[0:64, 3 * W + lo:3 * W + hi],
                                       wdb[0:128, bi * 128:bi * 128 + 64],
                                       pre4f[0:128, dx + lo:dx + hi], None))
                if t == 0:   # half1 y0 needs image row 63 (half0 parts)
                    for bi, dx in enumerate((-1, 0, 1)):
                        lo, hi = max(0, -dx), min(W, W - dx)
                        mms.insert(1, (pd[64:128, lo:hi],
                                       wdb[0:128, (3 + bi) * 128 + 64:(3 + bi) * 128 + 128],
                                       pre4f[0:128, 63 * W + dx + lo:63 * W + dx + hi],
                                       (0, 64)))
                for i, (o, l, r, tp) in enumerate(mms):
                    nc.tensor.matmul(o, l, r, start=(i == 0),
                                     stop=(i == len(mms) - 1),
                                     tile_position=tp)
                if t % 3 == 0:
                    nc.scalar.copy(v4f[:, base:base + NT], pd[:])
                else:
                    nc.vector.tensor_copy(v4f[:, base:base + NT], pd[:])
            # folded edge fixes (64-row halves, both halves via partition dim)
            for dy in (-1, 0, 1):
                ti_l = (dy + 1) * 3 + 0
                y0_, y1_ = max(0, 1 - dy), min(63, 64 - dy)
                out_ap = v4f[0:128, y0_ * W:y1_ * W + 1:W]
                src_ap = pre4f[0:128, (y0_ + dy) * W - 1:(y1_ + dy) * W:W]
                nc.vector.scalar_tensor_tensor(
                    out=out_ap, in0=src_ap, scalar=dwn4[0:128, ti_l:ti_l + 1],
                    in1=out_ap, op0=ALU.mult, op1=ALU.add)
                ti_r = (dy + 1) * 3 + 2
                y0_, y1_ = max(0, -1 - dy), min(63, 62 - dy)
                out_ap = v4f[0:128, y0_ * W + W - 1:y1_ * W + W:W]
                src_ap = pre4f[0:128, (y0_ + dy + 1) * W:(y1_ + dy + 1) * W + 1:W]
                nc.vector.scalar_tensor_tensor(
                    out=out_ap, in0=src_ap, scalar=dwn4[0:128, ti_r:ti_r + 1],
                    in1=out_ap, op0=ALU.mult, op1=ALU.add)

            for t in range(NTILES):
                h, lc = t // 16, (t % 16) * NT
                cols = slice(t * NT, (t + 1) * NT)
                lcols = slice(lc, lc + NT)
                p64 = slice(64 * h, 64 * h + 64)
                py0 = yps.tile([128, NT], F32, tag="py0")
                nc.tensor.matmul(py0[:], mt0[:, 0:128], v3[:, cols],
                                 start=True, stop=False)
                nc.tensor.matmul(py0[:], mt1[p64, 0:128], v4f[p64, lcols],
                                 start=False, stop=True,
                                 tile_position=(64 * h, 0))
                y0 = yp.tile([128, NT], BF16, tag="y0", bufs=6)
                if t % 2 == 0:
                    nc.vector.tensor_copy(y0[:], py0[:])
                else:
                    nc.scalar.copy(y0[:], py0[:])
                nc.sync.dma_start(y_d[0:128, cols], y0[:])
                py1 = yps.tile([64, NT], F32, tag="py1")
                nc.tensor.matmul(py1[:], mt0[:, 128:192], v3[:, cols],
                                 start=True, stop=False)
                nc.tensor.matmul(py1[:], mt1[p64, 128:192], v4f[p64, lcols],
                                 start=False, stop=True,
                                 tile_position=(64 * h, 0))
                y1 = yp.tile([64, NT], BF16, tag="y1", bufs=6)
                if t % 2 == 0:
                    nc.scalar.copy(y1[:], py1[:])
                else:
                    nc.vector.tensor_copy(y1[:], py1[:])
                nc.sync.dma_start(y_d[128:192, cols], y1[:])
            if yprobe_d is not None:
                nc.sync.dma_start(yprobe_d[:], y0[:, 0:4])

    nc.compile()
    return nc


def host_inputs(x, w_qkv, w_dw, w_out, temperature):
    """Host-side prep: per-core input maps."""
    b = x.shape[0]
    w_dw9 = np.asarray(w_dw, np.float32).reshape(576, 9)
    # fp8 pair-diag layout for q,k chunks 0..2
    wdp8 = np.zeros((3, 128, 10 * 128), np.float32)
    for ci in range(3):
        s, wid = CHUNKS[ci]
        for p, (tA, tB, _o, _s) in enumerate(PAIRS):
            for h, t in enumerate((tA, tB)):
                if t is None:
                    continue
                c0 = (2 * p + h) * 128
                wdp8[ci, :wid, c0:c0 + wid][np.arange(wid), np.arange(wid)] = \
                    w_dw9[s:s + wid, t]
    # bf16 single-tap diag layout: chunk3 plain; chunk4 folded block-diag2
    wdd = np.zeros((2, 128, 9 * 128), np.float32)
    for t in range(9):
        wdd[0, :128, t * 128:t * 128 + 128][np.arange(128), np.arange(128)] = \
            w_dw9[384:512, t]
        for half in (0, 1):
            o = 64 * half
            wdd[1, o:o + 64, t * 128 + o:t * 128 + o + 64][
                np.arange(64), np.arange(64)] = w_dw9[512:576, t]
    # folded boundary taps: blocks 0..2 dn (dy=+1), 3..5 up (dy=-1)
    wdb = np.zeros((128, 6 * 128), np.float32)
    for bi, t in enumerate((6, 7, 8)):   # (+1,-1),(+1,0),(+1,+1)
        wdb[64 + np.arange(64), bi * 128 + np.arange(64)] = w_dw9[512:576, t]
    for bi, t in enumerate((0, 1, 2)):   # (-1,-1),(-1,0),(-1,+1)
        wdb[np.arange(64), (3 + bi) * 128 + 64 + np.arange(64)] = \
            w_dw9[512:576, t]
    wdn = np.zeros((5, 128, 9), np.float32)
    for ci, (s, wid) in enumerate(CHUNKS):
        wdn[ci, :wid, :] = -w_dw9[s:s + wid, :]
    wdn[4, 64:128, :] = wdn[4, 0:64, :]  # folded: both halves
    wdq = np.ascontiguousarray(w_dw9[384:512, :])  # chunk3 positive taps
    temp_pc = np.repeat(np.asarray(temperature, np.float32).reshape(NHEADS),
                        HDIM).reshape(C, 1)
    mask = np.full((C, C), -1e9, np.float32)
    for h in range(NHEADS):
        mask[h * HDIM:(h + 1) * HDIM, h * HDIM:(h + 1) * HDIM] = 0.0
    wqT = np.ascontiguousarray(np.asarray(w_qkv, np.float32).T)
    wq8 = np.zeros((2, 128, 2, 576), np.float32)
    wq8[:, :, 0, :] = wqT[0:128]
    wq8[0, 0:64, 1, :] = wqT[128:192]
    wq8[1, 64:128, 1, :] = wqT[128:192]
    shared = {
        "w_qkvT": wqT.astype(ml_dtypes.bfloat16),
        "wq8": wq8.reshape(2, 128, 2 * 576).astype(ml_dtypes.float8_e4m3),
        "w_dwb": wdb.astype(ml_dtypes.bfloat16),
        "w_dwp8": wdp8.astype(ml_dtypes.float8_e4m3),
        "w_dwd": wdd.astype(ml_dtypes.bfloat16),
        "w_dwn": wdn,
        "w_dwq": wdq,
        "w_outT": np.ascontiguousarray(np.asarray(w_out, np.float32).T),
        "temp": temp_pc,
        "mask": mask,
        "eye8": np.eye(128, dtype=ml_dtypes.float8_e4m3),
        "eyef": np.eye(128, dtype=np.float32),
        "ones1": np.ones((1, 128), np.float32),
    }
    out_maps = []
    for c in range(b):
        xc = np.ascontiguousarray(np.asarray(x[c], np.float32).reshape(C, N))
        x8 = np.zeros((128, N + N // 2), np.float32)
        x8[:, 0:N] = xc[0:128]
        x8[0:64, N:N + N // 2] = xc[128:192, 0:N // 2]
        x8[64:128, N:N + N // 2] = xc[128:192, N // 2:N]
        out_maps.append(dict(shared, x=xc.astype(ml_dtypes.bfloat16),
                             x8=x8.astype(ml_dtypes.float8_e4m3)))
    return out_maps


_NC_CACHE = {}


def kernel(x, w_qkv, w_dw, w_out, temperature):
    x = np.asarray(x)
    if "nc" not in _NC_CACHE:
        _NC_CACHE["nc"] = build_nc()
    nc = _NC_CACHE["nc"]
    in_maps = host_inputs(x, w_qkv, w_dw, w_out, temperature)
    res = run_bass_kernel_spmd(nc, in_maps, list(range(8)))
    out = np.stack([np.asarray(res.results[c]["y"]).astype(np.float32)
                    .reshape(C, H, W) for c in range(8)])
    return out
